# revision 23
# baseline (speedup 1.0000x reference)
"""Distributed Trainium2 kernel for nn_Attention_64854006169830.

Strategy (8 NeuronCores, SPMD):
  - Head-parallel attention (core i owns head i), feature-major activations.
  - QKV projections in fp8-e4m3 DoubleRow mode (2x PE); weights pre-scaled
    by 32 on host (e4m3 normal range), scale folded out via the exp() scale
    and the SCL-valued ones-vector in the denominators.
  - Per-batch software pipeline: QKV(b+1) stages are woven into
    attention(b) emission so the PE fills the gaps while the ACT engine
    (softmax exp, the phase bottleneck) streams.
  - Softmax: exp on ACT writes shifted unnormalized probabilities straight
    to fp8; denominators and attn@V are fp8 DoubleRow matmuls; all
    SBUF-side transposes ride the DMA XBAR (dma_start_transpose), not PE.
  - ctx redistribution head-shard -> row-shard via per-batch AllToAll.
  - Row-parallel LN/FF/collapse in bf16, emitted as a 2-stage pipeline
    (head=LN1, body=FF+LN2) with c1 chunks interleaved to keep PE fed.
    FF bias is applied by a rank-1 ones-row matmul into the same PSUM
    accumulation, so ACT applies relu straight from PSUM.
  - Tail: per-core partial c @ wl1 (row-sharded), one bf16 AllReduce, then
    every core redundantly computes the tiny l2 matmul.
Compute dtype: bf16/fp8-e4m3 (f32 accumulation); ~0.6% rel err vs the
float32 reference (gate is 2e-2).
"""
import sys
import math

for _p in ("/opt/trn_rl_repo", "/opt/trn_rl_repo/concourse"):
    if _p not in sys.path:
        sys.path.insert(0, _p)

import numpy as np
import ml_dtypes

B, L, D, H, OUT = 4, 2048, 1024, 8, 256
DH = D // H          # 128
N = B * L            # 8192 rows
NC = 8               # cores
RPC = N // NC        # 1024 rows per core (as 4 batches x 256 L-positions)
LPC = L // NC        # 256 L-positions per core per batch
EPS = 1e-12
SCL = 32.0           # fp8 weight pre-scale (host); folded out on device
C_SHIFT = 2.0        # softmax exp shift; cancels in normalization
BIAS_MM = True       # ff bias via rank-1 matmul instead of DVE add

_CACHE = {}


def _build_nc(trivial_gb=False):
    import concourse.bass as bass
    import concourse.tile as tile
    from concourse import bacc, mybir
    from concourse.masks import make_identity

    BF = mybir.dt.bfloat16
    F8 = mybir.dt.float8e4
    F32 = mybir.dt.float32
    AF = mybir.ActivationFunctionType
    OP = mybir.AluOpType
    DR = mybir.MatmulPerfMode.DoubleRow

    nc = bacc.Bacc("TRN2", debug=False, num_devices=NC)

    # ---- parameters (per-core values supplied via in_maps) ----
    xT = nc.dram_tensor("xT", [D, N], F8, kind="ExternalInput")
    xrows = nc.dram_tensor("xrows", [RPC, D], BF, kind="ExternalInput")
    wqkv = nc.dram_tensor("wqkv", [3, D, DH], F8, kind="ExternalInput")
    bqkv = nc.dram_tensor("bqkv", [3, DH], F32, kind="ExternalInput")
    wff = nc.dram_tensor("wff", [D, D], BF, kind="ExternalInput")
    bff = nc.dram_tensor("bff", [D], BF, kind="ExternalInput")
    gamma = nc.dram_tensor("gamma", [D], BF, kind="ExternalInput")
    beta = nc.dram_tensor("beta", [D], BF, kind="ExternalInput")
    wc1 = nc.dram_tensor("wc1", [D, D], BF, kind="ExternalInput")   # gamma-folded
    bc1 = nc.dram_tensor("bc1", [D], F32, kind="ExternalInput")     # beta-folded
    wc2 = nc.dram_tensor("wc2", [D], BF, kind="ExternalInput")
    bc2 = nc.dram_tensor("bc2", [1], F32, kind="ExternalInput")
    wl1r = nc.dram_tensor("wl1r", [LPC, L], BF, kind="ExternalInput")  # row-shard
    bl1t4 = nc.dram_tensor("bl1t4", [L, B], BF, kind="ExternalInput")
    wl2f = nc.dram_tensor("wl2f", [L, OUT], BF, kind="ExternalInput")  # full
    bl2 = nc.dram_tensor("bl2", [OUT], F32, kind="ExternalInput")
    out = nc.dram_tensor("out", [B, OUT], F32, kind="ExternalOutput")

    # ---- internal DRAM ----
    a2a_in = [nc.dram_tensor(f"a2a_in{b}", [L, DH], BF) for b in range(B)]
    a2a_out = [nc.dram_tensor(f"a2a_out{b}", [L, DH], BF) for b in range(B)]
    sums_hbm = nc.dram_tensor("sums_hbm", [N], F32)
    c_hbm = nc.dram_tensor("c_hbm", [RPC], BF)
    l1p_in = [nc.dram_tensor(f"l1p_in{i}", [L, 2], BF) for i in range(2)]
    l1p_out = [nc.dram_tensor(f"l1p_out{i}", [L, 2], BF, addr_space="Shared")
               for i in range(2)]

    def bcast(dram_handle, parts, free):
        """Broadcast a [free] DRAM vector across `parts` partitions."""
        ap = dram_handle.ap()
        return bass.AP(tensor=ap.tensor, offset=0, ap=[[0, parts], [1, free]])

    RG = [list(range(NC))]
    KCB = L // 128   # 16 key chunks per batch
    DKC = D // 128   # 8
    ESC = 1.0 / (SCL * SCL)

    from contextlib import ExitStack

    with tile.TileContext(nc) as tc, ExitStack() as root:
        glob = root.enter_context(tc.tile_pool(name="glob", bufs=1))
        ones2 = glob.tile([128, 2, 128], F8)
        nc.vector.memset(ones2[:], SCL)  # folds the v-scale back out of ctx
        eps_sb = glob.tile([128, 1], F32)
        nc.vector.memset(eps_sb[:], EPS)
        negc_sb = glob.tile([128, 1], F32)
        nc.vector.memset(negc_sb[:], -C_SHIFT)
        ones_row = glob.tile([1, 128], BF)
        nc.vector.memset(ones_row[:], 1.0)
        ident1 = glob.tile([1, 1], F32)
        nc.vector.memset(ident1[:], 1.0)
        ident = glob.tile([128, 128], BF)
        make_identity(nc, ident[:])

        # Phase-C weight pool carved out first so its loads never overlap
        # (in address space) with the big transient phase-A/B tiles.
        wC_pool = root.enter_context(tc.tile_pool(name="wC", bufs=1))
        finW = root.enter_context(tc.tile_pool(name="finW", bufs=1))

        phAB = root.enter_context(ExitStack())
        qkv_pool = phAB.enter_context(tc.tile_pool(name="qkv", bufs=1))
        qkvT = [qkv_pool.tile([128, 2, L], BF, name=f"qkvT{b}") for b in range(B)]
        vnat = [qkv_pool.tile([128, L // 128, DH], F8, name=f"vnat{b}")
                for b in range(B)]

        xt_pool = phAB.enter_context(tc.tile_pool(name="xt", bufs=2))
        wq_pool = phAB.enter_context(tc.tile_pool(name="wqkv", bufs=1))
        vstage_pool = phAB.enter_context(tc.tile_pool(name="vstage", bufs=2))
        pT_pool = phAB.enter_context(tc.tile_pool(name="pT", bufs=2))
        ctxT_pool = phAB.enter_context(tc.tile_pool(name="ctxT", bufs=2))
        sums_pool = phAB.enter_context(tc.tile_pool(name="sums", bufs=1))
        recip_pool = phAB.enter_context(tc.tile_pool(name="recip", bufs=2))
        norm_pool = phAB.enter_context(tc.tile_pool(name="norm", bufs=3))
        psA = phAB.enter_context(tc.tile_pool(name="psA", bufs=1, space="PSUM"))
        psTr = phAB.enter_context(tc.tile_pool(name="psTr", bufs=1, space="PSUM"))
        psS = phAB.enter_context(tc.tile_pool(name="psS", bufs=2, space="PSUM"))
        psX = phAB.enter_context(tc.tile_pool(name="psX", bufs=2, space="PSUM"))

        wq_sb = wq_pool.tile([128, 3, DKC, DH], F8)
        nc.sync.dma_start(
            out=wq_sb[:],
            in_=bass.AP(tensor=wqkv.ap().tensor, offset=0,
                        ap=[[DH, 128], [D * DH, 3], [128 * DH, DKC], [1, DH]]))
        bq_sb = wq_pool.tile([128, 3], F32)
        nc.sync.dma_start(
            out=bq_sb[:],
            in_=bass.AP(tensor=bqkv.ap().tensor, offset=0,
                        ap=[[1, 128], [DH, 3]]))

        xt_tiles = {}
        xt_last_dma = None

        def emit_xt_load(b):
            xt = xt_pool.tile([128, DKC, L], F8, tag="xt", name=f"xt{b}")
            xt_tiles[b] = xt
            nonlocal xt_last_dma
            for kc in range(DKC):
                xt_last_dma = nc.sync.dma_start(
                    out=xt[:, kc, :],
                    in_=xT.ap()[kc * 128:(kc + 1) * 128,
                                b * 2048:(b + 1) * 2048])

        def emit_qkv_stage(b, s):
            """One projection (q/k/v) of batch b: 4 DR-matmul chains + epilogue."""
            xt = xt_tiles[b]
            for r4 in range(4):
                pst = psA.tile([128, 512], F32, tag="qkvps",
                               name=f"qkvps{b}_{s}_{r4}")
                for k2 in range(D // 256):
                    nc.tensor.matmul(
                        pst[:], wq_sb[:, s, 2 * k2:2 * k2 + 2, :],
                        xt[:, 2 * k2:2 * k2 + 2, r4 * 512:(r4 + 1) * 512],
                        start=(k2 == 0), stop=(k2 == D // 256 - 1),
                        perf_mode=DR)
                if s < 2:
                    nc.vector.tensor_scalar_add(
                        qkvT[b][:, s, r4 * 512:(r4 + 1) * 512], pst[:],
                        bq_sb[:, s:s + 1])
                else:
                    # v (bias folded into xrows on host): psum -> bf16 staging,
                    # XBAR transpose to row-major, then fp8 for the DR matmuls
                    vstage = vstage_pool.tile([128, 512], BF, tag="vstage",
                                              name=f"vst{b}_{r4}")
                    nc.vector.tensor_copy(vstage[:], pst[:])
                    vtr = psTr.tile([128, 4, 128], BF, tag="tr",
                                    name=f"vtr{b}_{r4}")
                    for j in range(4):
                        nc.tensor.transpose(vtr[:, j, :],
                                            vstage[:, j * 128:(j + 1) * 128],
                                            ident[:])
                    nc.vector.tensor_copy(vnat[b][:, r4 * 4:(r4 + 1) * 4, :], vtr[:])

        def emit_scores(b, qc, pT):
            q0 = qc * 1024
            for kc in range(KCB):
                sps = psS.tile([128, 1024], F32, tag="sps",
                               name=f"sps{b}_{qc}_{kc}")
                for hh in range(2):
                    nc.tensor.matmul(
                        sps[:, hh * 512:(hh + 1) * 512],
                        qkvT[b][:, 1, kc * 128:(kc + 1) * 128],
                        qkvT[b][:, 0, q0 + hh * 512: q0 + (hh + 1) * 512],
                        start=True, stop=True)
                nc.scalar.activation(pT[:, kc, :], sps[:], AF.Exp,
                                     bias=negc_sb[:], scale=ESC)

        def emit_scores_interleaved(b, qc, pT, pT_prev, cps_prev):
            """scores/exp for (b,qc) with ctx DR-matmuls of the previous qc
            woven in as the needed pT chunks complete."""
            q0 = qc * 1024
            for kc in range(KCB):
                sps = psS.tile([128, 1024], F32, tag="sps",
                               name=f"sps{b}_{qc}_{kc}i")
                for hh in range(2):
                    nc.tensor.matmul(
                        sps[:, hh * 512:(hh + 1) * 512],
                        qkvT[b][:, 1, kc * 128:(kc + 1) * 128],
                        qkvT[b][:, 0, q0 + hh * 512: q0 + (hh + 1) * 512],
                        start=True, stop=True)
                nc.scalar.activation(pT[:, kc, :], sps[:], AF.Exp,
                                     bias=negc_sb[:], scale=ESC)
                if kc % 2 == 1:
                    k2 = kc // 2
                    for hh in range(2):
                        nc.tensor.matmul(
                            cps_prev[hh][:],
                            vnat[b][:, 2 * k2:2 * k2 + 2, :],
                            pT_prev[:, 2 * k2:2 * k2 + 2,
                                    hh * 512:(hh + 1) * 512],
                            start=(k2 == 0), stop=(k2 == KCB // 2 - 1),
                            perf_mode=DR)

        def emit_ctx(b, pT, cps2):
            for k2 in range(KCB // 2):
                for hh in range(2):
                    nc.tensor.matmul(
                        cps2[hh][:], vnat[b][:, 2 * k2:2 * k2 + 2, :],
                        pT[:, 2 * k2:2 * k2 + 2, hh * 512:(hh + 1) * 512],
                        start=(k2 == 0), stop=(k2 == KCB // 2 - 1),
                        perf_mode=DR)

        def emit_den_epilogue(b, qc, pT, cps2, ctxT_sb):
            """ctxT copies, DR-ones denominators, recip roundtrip, XBAR
            transpose back to row-major, normalize, ship to a2a_in."""
            for hh in range(2):
                nc.vector.tensor_copy(
                    ctxT_sb[:, qc * 1024 + hh * 512: qc * 1024 + (hh + 1) * 512],
                    cps2[hh][:])
            sums_sb = sums_pool.tile([1, 1024], F32, tag="sums", bufs=2,
                                     name=f"sums{b}_{qc}")
            for hh in range(2):
                sps2 = psX.tile([128, 512], F32, tag="x", name=f"den{b}_{qc}_{hh}")
                for k2 in range(KCB // 2):
                    nc.tensor.matmul(sps2[:], ones2[:],
                                     pT[:, 2 * k2:2 * k2 + 2,
                                        hh * 512:(hh + 1) * 512],
                                     start=(k2 == 0), stop=(k2 == KCB // 2 - 1),
                                     perf_mode=DR)
                nc.vector.tensor_copy(sums_sb[:, hh * 512:(hh + 1) * 512],
                                      sps2[0:1, :])
            # transpose the denominator row back onto partitions on PE
            # (8 tiny [1,128] transposes; kills the HBM roundtrip)
            q_hbm = b * L + qc * 1024
            rcps = psX.tile([128, 8], F32, tag="x", name=f"rcps{b}_{qc}")
            for j in range(8):
                nc.tensor.transpose(rcps[:, j:j + 1],
                                    sums_sb[0:1, j * 128:(j + 1) * 128],
                                    ident1[:])
            rcols = recip_pool.tile([128, 8], F32, tag="rcols",
                                    name=f"rcols{b}_{qc}")
            nc.vector.reciprocal(rcols[:], rcps[:])
            ctr = psTr.tile([128, 8, 128], BF, tag="tr",
                            name=f"ctr{b}_{qc}")
            for j in range(8):
                nc.tensor.transpose(
                    ctr[:, j, :],
                    ctxT_sb[:, qc * 1024 + j * 128: qc * 1024 + (j + 1) * 128],
                    ident[:])
            nrm = norm_pool.tile([128, 8, DH], BF, tag="nrm",
                                 name=f"nrm{b}_{qc}")
            for j in range(8):
                nc.vector.tensor_scalar_mul(nrm[:, j, :], ctr[:, j, :],
                                            rcols[:, j:j + 1])
            nc.sync.dma_start(
                out=bass.AP(tensor=a2a_in[b].ap().tensor,
                            offset=qc * 1024 * DH,
                            ap=[[DH, 128], [128 * DH, 8], [1, DH]]),
                in_=nrm[:])

        # ---- phase-C weights live in root-carved pools; big loads are
        # emitted at batch boundaries so they never clog the SP queue ----
        wff_sb = wC_pool.tile([128, DKC, D], BF)
        wc1_sb = wC_pool.tile([128, DKC, D], BF)
        wc2_sb = wC_pool.tile([128, DKC], BF)
        bc1_sb = wC_pool.tile([128, DKC], F32)
        bc2_sb = wC_pool.tile([1, 1], F32)
        if not trivial_gb:
            gamma_bc = wC_pool.tile([128, D], BF)
            beta_bc = wC_pool.tile([128, D], BF)
        bffr = wC_pool.tile([1, D], BF)
        wl1r_sb = finW.tile([128, 2, L], BF)
        wl2_sb = finW.tile([128, L // 128, OUT], BF)
        bl1t_sb = finW.tile([128, L // 128, B], BF)
        bl2_bc = finW.tile([B, OUT], F32)

        def emit_small_weights():
            nc.sync.dma_start(
                out=wc2_sb[:],
                in_=bass.AP(tensor=wc2.ap().tensor, offset=0,
                            ap=[[1, 128], [128, DKC]]))
            nc.sync.dma_start(
                out=bc1_sb[:],
                in_=bass.AP(tensor=bc1.ap().tensor, offset=0,
                            ap=[[1, 128], [128, DKC]]))
            nc.sync.dma_start(out=bc2_sb[:], in_=bc2.ap())
            if not trivial_gb:
                nc.sync.dma_start(out=gamma_bc[:], in_=bcast(gamma, 128, D))
                nc.sync.dma_start(out=beta_bc[:], in_=bcast(beta, 128, D))
            nc.sync.dma_start(out=bffr[:],
                              in_=bff.ap().rearrange("(o n) -> o n", o=1))
            nc.sync.dma_start(
                out=bl1t_sb[:],
                in_=bass.AP(tensor=bl1t4.ap().tensor, offset=0,
                            ap=[[B, 128], [128 * B, L // 128], [1, B]]))
            nc.sync.dma_start(out=bl2_bc[:], in_=bcast(bl2, B, OUT))

        def emit_big_weights(stage):
            if stage == 1:
                nc.sync.dma_start(
                    out=wff_sb[:],
                    in_=bass.AP(tensor=wff.ap().tensor, offset=0,
                                ap=[[D, 128], [128 * D, DKC], [1, D]]))
            elif stage == 2:
                nc.sync.dma_start(
                    out=wc1_sb[:],
                    in_=bass.AP(tensor=wc1.ap().tensor, offset=0,
                                ap=[[D, 128], [128 * D, DKC], [1, D]]))
            elif stage == 3:
                nc.sync.dma_start(
                    out=wl1r_sb[:],
                    in_=bass.AP(tensor=wl1r.ap().tensor, offset=0,
                                ap=[[L, 128], [128 * L, 2], [1, L]]))
                nc.sync.dma_start(
                    out=wl2_sb[:],
                    in_=bass.AP(tensor=wl2f.ap().tensor, offset=0,
                                ap=[[OUT, 128], [128 * OUT, L // 128], [1, OUT]]))

        # ---------------- merged phase A+B, per-batch pipeline ----------------
        emit_xt_load(0)
        emit_small_weights()
        for s in range(3):
            emit_qkv_stage(0, s)
        for b in range(B):
            if b + 1 < B:
                emit_xt_load(b + 1)
            if 1 <= b <= 3:
                emit_big_weights(b)
            ctxT_sb = ctxT_pool.tile([128, L], BF, tag="ctxT", name=f"ctxT{b}")
            pT0 = pT_pool.tile([128, KCB, 1024], F8, tag="pT", name=f"pT{b}_0")
            emit_scores(b, 0, pT0)
            if b + 1 < B:
                emit_qkv_stage(b + 1, 0)
            pT1 = pT_pool.tile([128, KCB, 1024], F8, tag="pT", name=f"pT{b}_1")
            cps2_q0 = [psX.tile([128, 512], F32, tag="x", name=f"cps{b}_0_{h}")
                       for h in range(2)]
            emit_scores_interleaved(b, 1, pT1, pT0, cps2_q0)
            if b + 1 < B:
                emit_qkv_stage(b + 1, 1)
            emit_den_epilogue(b, 0, pT0, cps2_q0, ctxT_sb)
            if b + 1 < B:
                emit_qkv_stage(b + 1, 2)
            cps2_q1 = [psX.tile([128, 512], F32, tag="x", name=f"cps{b}_1_{h}")
                       for h in range(2)]
            emit_ctx(b, pT1, cps2_q1)
            emit_den_epilogue(b, 1, pT1, cps2_q1, ctxT_sb)
            nc.gpsimd.collective_compute(
                "AllToAll", OP.bypass,
                ins=[a2a_in[b].ap()],
                outs=[a2a_out[b].ap()],
                replica_groups=RG)

        phAB.close()  # release qkv/pT space for phase C

        # ================= Phase C: row-parallel LN/FF/collapse =================
        with ExitStack() as phC:
            rowC = phC.enter_context(tc.tile_pool(name="rowC", bufs=4))
            h2T_pool = phC.enter_context(tc.tile_pool(name="h2T", bufs=1))
            psFF = phC.enter_context(tc.tile_pool(name="psFF", bufs=2, space="PSUM"))
            psC1 = phC.enter_context(tc.tile_pool(name="psC1", bufs=2, space="PSUM"))
            psTrC = phC.enter_context(tc.tile_pool(name="psTrC", bufs=2, space="PSUM"))
            psSm = phC.enter_context(tc.tile_pool(name="psSm", bufs=2, space="PSUM"))

            h2T_half = [h2T_pool.tile([128, DKC, RPC // 2], BF, name=f"h2Th{i}")
                        for i in range(2)]
            c2_sb = h2T_pool.tile([1, RPC], F32)
            c1T = h2T_pool.tile([128, DKC, RPC], BF)

            def layernorm_rows(src, dst, apply_gb):
                stats = rowC.tile([128, 2, nc.vector.BN_STATS_DIM], F32, tag="stats")
                for sg in range(2):
                    nc.vector.bn_stats(stats[:, sg, :], src[:, sg * 512:(sg + 1) * 512])
                mv = rowC.tile([128, nc.vector.BN_AGGR_DIM], F32, tag="mv")
                nc.vector.bn_aggr(mv[:], stats[:])
                sq = rowC.tile([128, 1], F32, tag="sq")
                nc.scalar.activation(sq[:], mv[:, 1:2], AF.Sqrt, bias=eps_sb[:], scale=1.0)
                rstd = rowC.tile([128, 1], F32, tag="rstd")
                nc.vector.reciprocal(rstd[:], sq[:])
                if apply_gb and not trivial_gb:
                    z = rowC.tile([128, D], BF, tag="zf")
                    nc.vector.tensor_scalar(z[:], src[:], mv[:, 0:1], rstd[:],
                                            op0=OP.subtract, op1=OP.mult)
                    zg = rowC.tile([128, D], BF, tag="zg")
                    nc.vector.tensor_mul(zg[:], z[:], gamma_bc[:])
                    nc.vector.tensor_add(dst[:], zg[:], beta_bc[:])
                else:
                    nc.vector.tensor_scalar(dst[:], src[:], mv[:, 0:1], rstd[:],
                                            op0=OP.subtract, op1=OP.mult)

            h1b_t = {}
            h1T_t = {}

            def emit_head(t):
                """loads + residual add + LN1 + XBAR transpose of h1."""
                b, e = t // 2, t % 2
                ctx_t = rowC.tile([128, H, DH], BF, tag="ctx_t")
                nc.sync.dma_start(
                    out=ctx_t[:],
                    in_=bass.AP(tensor=a2a_out[b].ap().tensor,
                                offset=e * 128 * DH,
                                ap=[[DH, 128], [LPC * DH, H], [1, DH]]))
                x_t = rowC.tile([128, D], BF, tag="x_t")
                nc.sync.dma_start(out=x_t[:], in_=xrows.ap()[t * 128:(t + 1) * 128, :])
                s_t = rowC.tile([128, D], BF, tag="s_t")
                nc.vector.tensor_add(s_t[:], x_t[:],
                                     ctx_t[:].rearrange("p h d -> p (h d)"))
                h1b = rowC.tile([128, D], BF, tag="h1b")
                layernorm_rows(s_t, h1b, apply_gb=True)
                h1T = rowC.tile([128, DKC, 128], BF, tag="h1T")
                if t < 4:
                    tps1 = psTrC.tile([128, DKC, 128], BF, tag="htr",
                                      name=f"h1tr{t}")
                    for kc in range(DKC):
                        nc.tensor.transpose(tps1[:, kc, :],
                                            h1b[:, kc * 128:(kc + 1) * 128],
                                            ident[:])
                    nc.vector.tensor_copy(h1T[:], tps1[:])
                else:
                    nc.scalar.dma_start_transpose(out=h1T[:], in_=h1b[:])
                h1b_t[t] = h1b
                h1T_t[t] = h1T

            def emit_body(t):
                """ff + residual + LN2 + XBAR transpose of h2 into h2T_half."""
                h1b, h1T = h1b_t.pop(t), h1T_t.pop(t)
                f_t = rowC.tile([128, D], BF, tag="f_t")
                for dc in range(2):
                    fps = psFF.tile([128, 512], F32, tag="fps")
                    for kc in range(DKC):
                        nc.tensor.matmul(fps[:], h1T[:, kc, :],
                                         wff_sb[:, kc, dc * 512:(dc + 1) * 512],
                                         start=(kc == 0), stop=False)
                    nc.tensor.matmul(fps[:], ones_row[:],
                                     bffr[0:1, dc * 512:(dc + 1) * 512],
                                     start=False, stop=True)
                    nc.scalar.activation(f_t[:, dc * 512:(dc + 1) * 512], fps[:],
                                         AF.Relu, bias=0.0, scale=1.0)
                s2_t = rowC.tile([128, D], BF, tag="s2_t")
                nc.vector.tensor_add(s2_t[:], h1b[:], f_t[:])
                h2b = rowC.tile([128, D], BF, tag="h2b")
                layernorm_rows(s2_t, h2b, apply_gb=False)
                if t < 4:
                    tps2 = psTrC.tile([128, DKC, 128], BF, tag="htr",
                                      name=f"h2tr{t}")
                    for kc in range(DKC):
                        nc.tensor.transpose(tps2[:, kc, :],
                                            h2b[:, kc * 128:(kc + 1) * 128],
                                            ident[:])
                    nc.vector.tensor_copy(
                        h2T_half[t // 4][:, :, (t % 4) * 128:(t % 4 + 1) * 128],
                        tps2[:])
                else:
                    nc.scalar.dma_start_transpose(
                        out=h2T_half[t // 4][:, :, (t % 4) * 128:(t % 4 + 1) * 128],
                        in_=h2b[:])

            def emit_c1(rc, fc):
                cps = psC1.tile([128, 512], F32, tag="c1ps", name=f"c1ps{rc}_{fc}")
                for kc in range(DKC):
                    nc.tensor.matmul(cps[:], wc1_sb[:, kc, fc * 128:(fc + 1) * 128],
                                     h2T_half[rc][:, kc, :],
                                     start=(kc == 0), stop=(kc == DKC - 1))
                nc.scalar.activation(c1T[:, fc, rc * 512:(rc + 1) * 512], cps[:],
                                     AF.Relu, bias=bc1_sb[:, fc:fc + 1], scale=1.0)

            def emit_c2(rc):
                c2ps = psSm.tile([1, 512], F32, tag="sm", name=f"c2ps{rc}")
                for kc in range(DKC):
                    nc.tensor.matmul(c2ps[:], wc2_sb[:, kc:kc + 1],
                                     c1T[:, kc, rc * 512:(rc + 1) * 512],
                                     start=(kc == 0), stop=(kc == DKC - 1))
                nc.scalar.activation(c2_sb[0:1, rc * 512:(rc + 1) * 512], c2ps[:],
                                     AF.Relu, bias=bc2_sb[0:1, :], scale=1.0)

            # 2-stage pipeline with c1 chunks as PE filler
            emit_head(0)
            emit_head(1)
            c1_sched = {3: [(0, 0), (0, 1)], 4: [(0, 2), (0, 3)],
                        5: [(0, 4), (0, 5)], 6: [(0, 6), (0, 7)],
                        7: [(1, 0), (1, 1)]}
            for t in range(RPC // 128):
                emit_body(t)
                if t + 2 < RPC // 128:
                    emit_head(t + 2)
                for rc, fc in c1_sched.get(t, []):
                    emit_c1(rc, fc)
            # ---- tail: per-half partial c @ wl1 (batches 01 then 23), two
            # overlapped AllReduces, then the tiny l2 matmul on every core ----
            def emit_l1_half(i):
                # half i covers c2 rows rc=i (batches 2i, 2i+1)
                ctps = psSm.tile([128, 2, 2], F32, tag="sm", name=f"ctps{i}")
                for bb in range(2):
                    for e in range(2):
                        nc.tensor.transpose(
                            ctps[:, e, bb:bb + 1],
                            c2_sb[0:1, ((2 * i + bb) * 2 + e) * 128:
                                       ((2 * i + bb) * 2 + e + 1) * 128],
                            ident1[:])
                cT_sb = rowC.tile([128, 2, 2], BF, tag="cT_sb",
                                  name=f"cT{i}")
                nc.vector.tensor_copy(cT_sb[:], ctps[:])
                l1ps = psSm.tile([128, L // 128, 2], F32, tag="sm",
                                 name=f"l1ps{i}")
                for j in range(L // 128):
                    for e in range(2):
                        nc.tensor.matmul(l1ps[:, j, :],
                                         wl1r_sb[:, e, j * 128:(j + 1) * 128],
                                         cT_sb[:, e, :],
                                         start=(e == 0), stop=(e == 1))
                l1p_sb = rowC.tile([128, L // 128, 2], BF, tag="l1p_sb",
                                   name=f"l1p{i}")
                nc.vector.tensor_copy(l1p_sb[:], l1ps[:])
                nc.sync.dma_start(
                    out=bass.AP(tensor=l1p_in[i].ap().tensor, offset=0,
                                ap=[[2, 128], [256, L // 128], [1, 2]]),
                    in_=l1p_sb[:])
                nc.gpsimd.collective_compute(
                    "AllReduce", OP.add,
                    ins=[l1p_in[i].ap()], outs=[l1p_out[i].ap()],
                    replica_groups=RG)

            emit_c2(0)
            emit_l1_half(0)
            for fc in range(2, DKC):
                emit_c1(1, fc)
            emit_c2(1)
            emit_l1_half(1)

            c1fT = rowC.tile([128, L // 128, B], BF, tag="c1fT")
            for i in range(2):
                arT_sb = rowC.tile([128, L // 128, 2], BF, tag="arT_sb",
                                   name=f"arT{i}")
                nc.sync.dma_start(
                    out=arT_sb[:],
                    in_=bass.AP(tensor=l1p_out[i].ap().tensor, offset=0,
                                ap=[[2, 128], [256, L // 128], [1, 2]]))
                l1b_sb = rowC.tile([128, L // 128, 2], BF, tag="l1b_sb",
                                   name=f"l1b{i}")
                nc.vector.tensor_add(l1b_sb[:], arT_sb[:],
                                     bl1t_sb[:, :, 2 * i:2 * i + 2])
                nc.vector.tensor_scalar_max(c1fT[:, :, 2 * i:2 * i + 2],
                                            l1b_sb[:], 0.0)
            ops = psSm.tile([B, OUT], F32, tag="sm", name="finps")
            for j in range(L // 128):
                nc.tensor.matmul(ops[:], c1fT[:, j, :], wl2_sb[:, j, :],
                                 start=(j == 0), stop=(j == L // 128 - 1))
            out_f = rowC.tile([B, OUT], F32, tag="out_f")
            nc.vector.tensor_add(out_f[:], ops[:], bl2_bc[:])
            nc.sync.dma_start(out=out.ap(), in_=out_f[:])

    nc.compile()
    return nc


def _to_bf16(a):
    return np.asarray(a, dtype=np.float32).astype(ml_dtypes.bfloat16)


def _to_f8(a):
    return np.asarray(a, dtype=np.float32).astype(ml_dtypes.float8_e4m3)


def kernel(**inputs):
    from concourse.bass_utils import run_bass_kernel_spmd

    gamma_np0 = np.asarray(inputs["gamma"], dtype=np.float32)
    beta_np0 = np.asarray(inputs["beta"], dtype=np.float32)
    trivial_gb = bool(np.all(gamma_np0 == 1.0) and np.all(beta_np0 == 0.0))
    key = ("nc", trivial_gb)
    if key not in _CACHE:
        _CACHE[key] = _build_nc(trivial_gb=trivial_gb)
    nc = _CACHE[key]

    x = np.asarray(inputs["x"], dtype=np.float32).reshape(N, D)
    isq = 1.0 / math.sqrt(DH)
    gamma_np = np.asarray(inputs["gamma"], dtype=np.float32)
    beta_np = np.asarray(inputs["beta"], dtype=np.float32)
    wc1_np = np.asarray(inputs["wc1"], dtype=np.float32)
    bc1_np = np.asarray(inputs["bc1"], dtype=np.float32)
    # fold LN2's gamma/beta into the c1 projection (h2 feeds only this matmul)
    wc1_f = gamma_np[:, None] * wc1_np
    bc1_f = bc1_np + beta_np @ wc1_np

    xT_f8 = np.ascontiguousarray(_to_f8(x).T)
    shared = dict(
        xT=xT_f8,
        wff=_to_bf16(inputs["wff"]),
        bff=_to_bf16(inputs["bff"]),
        gamma=_to_bf16(gamma_np), beta=_to_bf16(beta_np),
        wc1=_to_bf16(wc1_f), bc1=bc1_f.astype(np.float32),
        wc2=_to_bf16(np.asarray(inputs["wc2"]).reshape(D)),
        bc2=np.asarray(inputs["bc2"], np.float32).reshape(1),
        bl2=np.asarray(inputs["bl2"], np.float32),
    )
    wl1_np = np.asarray(inputs["wl1"], np.float32)
    bl1_np = np.asarray(inputs["bl1"], np.float32)
    wl2_np = np.asarray(inputs["wl2"], np.float32)
    shared["bl1t4"] = _to_bf16(np.repeat(bl1_np[:, None], B, axis=1))
    shared["wl2f"] = _to_bf16(wl2_np)
    # fp8 weights pre-scaled by SCL so they sit in e4m3's normal range;
    # the scale is undone by ESC in exp() and the SCL-valued ones-vector
    wq = np.asarray(inputs["wq"], np.float32) * (isq * SCL)
    bq = np.asarray(inputs["bq"], np.float32) * (isq * SCL)
    wk = np.asarray(inputs["wk"], np.float32) * SCL
    bk = np.asarray(inputs["bk"], np.float32) * SCL
    wv = np.asarray(inputs["wv"], np.float32) * SCL
    bv = np.asarray(inputs["bv"], np.float32)

    in_maps = []
    for i in range(NC):
        sl = slice(i * DH, (i + 1) * DH)
        wqkv_i = np.stack([wq[:, sl], wk[:, sl], wv[:, sl]])
        bqkv_i = np.stack([bq[sl], bk[sl], np.zeros_like(bk[sl])])
        # rows this core owns after the A2A; v-bias folded into x here
        xr = np.concatenate([
            x[b * L + i * LPC: b * L + (i + 1) * LPC, :] for b in range(B)
        ]) + bv[None, :]
        in_maps.append(dict(
            shared,
            wqkv=_to_f8(wqkv_i),
            bqkv=bqkv_i.astype(np.float32),
            xrows=_to_bf16(xr),
            wl1r=_to_bf16(wl1_np[i * LPC:(i + 1) * LPC, :]),
        ))

    res = run_bass_kernel_spmd(nc, in_maps, core_ids=list(range(NC)))
    return np.asarray(res.results[0]["out"], dtype=np.float32)


# revision 25
# speedup vs baseline: 1.0200x; 1.0200x over previous
"""Distributed Trainium2 kernel for nn_Attention_64854006169830.

Strategy (8 NeuronCores, SPMD):
  - Head-parallel attention (core i owns head i), feature-major activations.
  - QKV projections in fp8-e4m3 DoubleRow mode (2x PE); weights pre-scaled
    by 32 on host (e4m3 normal range), scale folded out via the exp() scale
    and the SCL-valued ones-vector in the denominators.
  - Per-batch software pipeline: QKV(b+1) stages are woven into
    attention(b) emission so the PE fills the gaps while the ACT engine
    (softmax exp, the phase bottleneck) streams.
  - Softmax: exp on ACT writes shifted unnormalized probabilities straight
    to fp8; denominators and attn@V are fp8 DoubleRow matmuls; all
    SBUF-side transposes ride the DMA XBAR (dma_start_transpose), not PE.
  - ctx redistribution head-shard -> row-shard via per-batch AllToAll.
  - Row-parallel LN/FF/collapse in bf16, emitted as a 2-stage pipeline
    (head=LN1, body=FF+LN2) with c1 chunks interleaved to keep PE fed.
    FF bias is applied by a rank-1 ones-row matmul into the same PSUM
    accumulation, so ACT applies relu straight from PSUM.
  - Tail: per-core partial c @ wl1 (row-sharded), one bf16 AllReduce, then
    every core redundantly computes the tiny l2 matmul.
Compute dtype: bf16/fp8-e4m3 (f32 accumulation); ~0.6% rel err vs the
float32 reference (gate is 2e-2).
"""
import sys
import math

for _p in ("/opt/trn_rl_repo", "/opt/trn_rl_repo/concourse"):
    if _p not in sys.path:
        sys.path.insert(0, _p)

import numpy as np
import ml_dtypes

B, L, D, H, OUT = 4, 2048, 1024, 8, 256
DH = D // H          # 128
N = B * L            # 8192 rows
NC = 8               # cores
RPC = N // NC        # 1024 rows per core (as 4 batches x 256 L-positions)
LPC = L // NC        # 256 L-positions per core per batch
EPS = 1e-12
SCL = 32.0           # fp8 weight pre-scale (host); folded out on device
C_SHIFT = 2.0        # softmax exp shift; cancels in normalization
BIAS_MM = True       # ff bias via rank-1 matmul instead of DVE add

_CACHE = {}


def _build_nc(trivial_gb=False):
    import concourse.bass as bass
    import concourse.tile as tile
    from concourse import bacc, mybir
    from concourse.masks import make_identity

    BF = mybir.dt.bfloat16
    F8 = mybir.dt.float8e4
    F32 = mybir.dt.float32
    AF = mybir.ActivationFunctionType
    OP = mybir.AluOpType
    DR = mybir.MatmulPerfMode.DoubleRow

    nc = bacc.Bacc("TRN2", debug=False, num_devices=NC)

    # ---- parameters (per-core values supplied via in_maps) ----
    xT = nc.dram_tensor("xT", [D, N], F8, kind="ExternalInput")
    xrows = nc.dram_tensor("xrows", [RPC, D], BF, kind="ExternalInput")
    wqkv = nc.dram_tensor("wqkv", [3, D, DH], F8, kind="ExternalInput")
    bqkv = nc.dram_tensor("bqkv", [3, DH], F32, kind="ExternalInput")
    wff = nc.dram_tensor("wff", [D, D], BF, kind="ExternalInput")
    bff = nc.dram_tensor("bff", [D], BF, kind="ExternalInput")
    gamma = nc.dram_tensor("gamma", [D], BF, kind="ExternalInput")
    beta = nc.dram_tensor("beta", [D], BF, kind="ExternalInput")
    wc1 = nc.dram_tensor("wc1", [D, D], BF, kind="ExternalInput")   # gamma-folded
    bc1 = nc.dram_tensor("bc1", [D], F32, kind="ExternalInput")     # beta-folded
    wc2 = nc.dram_tensor("wc2", [D], BF, kind="ExternalInput")
    bc2 = nc.dram_tensor("bc2", [1], F32, kind="ExternalInput")
    wl1r = nc.dram_tensor("wl1r", [LPC, L], BF, kind="ExternalInput")  # row-shard
    bl1t4 = nc.dram_tensor("bl1t4", [L, B], BF, kind="ExternalInput")
    wl2f = nc.dram_tensor("wl2f", [L, OUT], BF, kind="ExternalInput")  # full
    bl2 = nc.dram_tensor("bl2", [OUT], F32, kind="ExternalInput")
    out = nc.dram_tensor("out", [B, OUT], F32, kind="ExternalOutput")

    # ---- internal DRAM ----
    a2a_in = [nc.dram_tensor(f"a2a_in{b}", [L, DH], BF) for b in range(B)]
    a2a_out = [nc.dram_tensor(f"a2a_out{b}", [L, DH], BF) for b in range(B)]
    sums_hbm = nc.dram_tensor("sums_hbm", [N], F32)
    c_hbm = nc.dram_tensor("c_hbm", [RPC], BF)
    l1p_in = nc.dram_tensor("l1p_in", [L, B], BF)
    l1p_out = nc.dram_tensor("l1p_out", [L, B], BF, addr_space="Shared")

    def bcast(dram_handle, parts, free):
        """Broadcast a [free] DRAM vector across `parts` partitions."""
        ap = dram_handle.ap()
        return bass.AP(tensor=ap.tensor, offset=0, ap=[[0, parts], [1, free]])

    RG = [list(range(NC))]
    KCB = L // 128   # 16 key chunks per batch
    DKC = D // 128   # 8
    ESC = 1.0 / (SCL * SCL)

    from contextlib import ExitStack

    with tile.TileContext(nc) as tc, ExitStack() as root:
        glob = root.enter_context(tc.tile_pool(name="glob", bufs=1))
        ones2 = glob.tile([128, 2, 128], F8)
        nc.vector.memset(ones2[:], SCL)  # folds the v-scale back out of ctx
        eps_sb = glob.tile([128, 1], F32)
        nc.vector.memset(eps_sb[:], EPS)
        negc_sb = glob.tile([128, 1], F32)
        nc.vector.memset(negc_sb[:], -C_SHIFT)
        ones_row = glob.tile([1, 128], BF)
        nc.vector.memset(ones_row[:], 1.0)
        ident1 = glob.tile([1, 1], F32)
        nc.vector.memset(ident1[:], 1.0)
        ident = glob.tile([128, 128], BF)
        make_identity(nc, ident[:])

        # Phase-C weight pool carved out first so its loads never overlap
        # (in address space) with the big transient phase-A/B tiles.
        wC_pool = root.enter_context(tc.tile_pool(name="wC", bufs=1))
        finW = root.enter_context(tc.tile_pool(name="finW", bufs=1))

        phAB = root.enter_context(ExitStack())
        qkv_pool = phAB.enter_context(tc.tile_pool(name="qkv", bufs=1))
        qkvT = [qkv_pool.tile([128, 2, L], BF, name=f"qkvT{b}") for b in range(B)]
        vnat = [qkv_pool.tile([128, L // 128, DH], F8, name=f"vnat{b}")
                for b in range(B)]

        xt_pool = phAB.enter_context(tc.tile_pool(name="xt", bufs=2))
        wq_pool = phAB.enter_context(tc.tile_pool(name="wqkv", bufs=1))
        vstage_pool = phAB.enter_context(tc.tile_pool(name="vstage", bufs=2))
        pT_pool = phAB.enter_context(tc.tile_pool(name="pT", bufs=2))
        ctxT_pool = phAB.enter_context(tc.tile_pool(name="ctxT", bufs=2))
        sums_pool = phAB.enter_context(tc.tile_pool(name="sums", bufs=1))
        recip_pool = phAB.enter_context(tc.tile_pool(name="recip", bufs=2))
        norm_pool = phAB.enter_context(tc.tile_pool(name="norm", bufs=3))
        psA = phAB.enter_context(tc.tile_pool(name="psA", bufs=1, space="PSUM"))
        psTr = phAB.enter_context(tc.tile_pool(name="psTr", bufs=1, space="PSUM"))
        psS = phAB.enter_context(tc.tile_pool(name="psS", bufs=2, space="PSUM"))
        psX = phAB.enter_context(tc.tile_pool(name="psX", bufs=2, space="PSUM"))

        wq_sb = wq_pool.tile([128, 3, DKC, DH], F8)
        nc.sync.dma_start(
            out=wq_sb[:],
            in_=bass.AP(tensor=wqkv.ap().tensor, offset=0,
                        ap=[[DH, 128], [D * DH, 3], [128 * DH, DKC], [1, DH]]))
        bq_sb = wq_pool.tile([128, 3], F32)
        nc.sync.dma_start(
            out=bq_sb[:],
            in_=bass.AP(tensor=bqkv.ap().tensor, offset=0,
                        ap=[[1, 128], [DH, 3]]))

        xt_tiles = {}
        xt_last_dma = None

        def emit_xt_load(b):
            xt = xt_pool.tile([128, DKC, L], F8, tag="xt", name=f"xt{b}")
            xt_tiles[b] = xt
            nonlocal xt_last_dma
            for kc in range(DKC):
                xt_last_dma = nc.sync.dma_start(
                    out=xt[:, kc, :],
                    in_=xT.ap()[kc * 128:(kc + 1) * 128,
                                b * 2048:(b + 1) * 2048])

        def emit_qkv_stage(b, s):
            """One projection (q/k/v) of batch b: 4 DR-matmul chains + epilogue."""
            xt = xt_tiles[b]
            for r4 in range(4):
                pst = psA.tile([128, 512], F32, tag="qkvps",
                               name=f"qkvps{b}_{s}_{r4}")
                for k2 in range(D // 256):
                    nc.tensor.matmul(
                        pst[:], wq_sb[:, s, 2 * k2:2 * k2 + 2, :],
                        xt[:, 2 * k2:2 * k2 + 2, r4 * 512:(r4 + 1) * 512],
                        start=(k2 == 0), stop=(k2 == D // 256 - 1),
                        perf_mode=DR)
                if s < 2:
                    nc.vector.tensor_scalar_add(
                        qkvT[b][:, s, r4 * 512:(r4 + 1) * 512], pst[:],
                        bq_sb[:, s:s + 1])
                else:
                    # v (bias folded into xrows on host): psum -> bf16 staging,
                    # XBAR transpose to row-major, then fp8 for the DR matmuls
                    vstage = vstage_pool.tile([128, 512], BF, tag="vstage",
                                              name=f"vst{b}_{r4}")
                    nc.vector.tensor_copy(vstage[:], pst[:])
                    vtr = psTr.tile([128, 4, 128], BF, tag="tr",
                                    name=f"vtr{b}_{r4}")
                    for j in range(4):
                        nc.tensor.transpose(vtr[:, j, :],
                                            vstage[:, j * 128:(j + 1) * 128],
                                            ident[:])
                    nc.vector.tensor_copy(vnat[b][:, r4 * 4:(r4 + 1) * 4, :], vtr[:])

        def emit_scores(b, qc, pT):
            q0 = qc * 1024
            for kc in range(KCB):
                sps = psS.tile([128, 1024], F32, tag="sps",
                               name=f"sps{b}_{qc}_{kc}")
                for hh in range(2):
                    nc.tensor.matmul(
                        sps[:, hh * 512:(hh + 1) * 512],
                        qkvT[b][:, 1, kc * 128:(kc + 1) * 128],
                        qkvT[b][:, 0, q0 + hh * 512: q0 + (hh + 1) * 512],
                        start=True, stop=True)
                nc.scalar.activation(pT[:, kc, :], sps[:], AF.Exp,
                                     bias=negc_sb[:], scale=ESC)

        def emit_scores_interleaved(b, qc, pT, pT_prev, cps_prev):
            """scores/exp for (b,qc) with ctx DR-matmuls of the previous qc
            woven in as the needed pT chunks complete."""
            q0 = qc * 1024
            for kc in range(KCB):
                sps = psS.tile([128, 1024], F32, tag="sps",
                               name=f"sps{b}_{qc}_{kc}i")
                for hh in range(2):
                    nc.tensor.matmul(
                        sps[:, hh * 512:(hh + 1) * 512],
                        qkvT[b][:, 1, kc * 128:(kc + 1) * 128],
                        qkvT[b][:, 0, q0 + hh * 512: q0 + (hh + 1) * 512],
                        start=True, stop=True)
                nc.scalar.activation(pT[:, kc, :], sps[:], AF.Exp,
                                     bias=negc_sb[:], scale=ESC)
                if kc % 2 == 1:
                    k2 = kc // 2
                    for hh in range(2):
                        nc.tensor.matmul(
                            cps_prev[hh][:],
                            vnat[b][:, 2 * k2:2 * k2 + 2, :],
                            pT_prev[:, 2 * k2:2 * k2 + 2,
                                    hh * 512:(hh + 1) * 512],
                            start=(k2 == 0), stop=(k2 == KCB // 2 - 1),
                            perf_mode=DR)

        def emit_ctx(b, pT, cps2):
            for k2 in range(KCB // 2):
                for hh in range(2):
                    nc.tensor.matmul(
                        cps2[hh][:], vnat[b][:, 2 * k2:2 * k2 + 2, :],
                        pT[:, 2 * k2:2 * k2 + 2, hh * 512:(hh + 1) * 512],
                        start=(k2 == 0), stop=(k2 == KCB // 2 - 1),
                        perf_mode=DR)

        def emit_den_epilogue(b, qc, pT, cps2, ctxT_sb):
            """ctxT copies, DR-ones denominators, recip roundtrip, XBAR
            transpose back to row-major, normalize, ship to a2a_in."""
            for hh in range(2):
                nc.vector.tensor_copy(
                    ctxT_sb[:, qc * 1024 + hh * 512: qc * 1024 + (hh + 1) * 512],
                    cps2[hh][:])
            sums_sb = sums_pool.tile([1, 1024], F32, tag="sums", bufs=2,
                                     name=f"sums{b}_{qc}")
            for hh in range(2):
                sps2 = psX.tile([128, 512], F32, tag="x", name=f"den{b}_{qc}_{hh}")
                for k2 in range(KCB // 2):
                    nc.tensor.matmul(sps2[:], ones2[:],
                                     pT[:, 2 * k2:2 * k2 + 2,
                                        hh * 512:(hh + 1) * 512],
                                     start=(k2 == 0), stop=(k2 == KCB // 2 - 1),
                                     perf_mode=DR)
                nc.vector.tensor_copy(sums_sb[:, hh * 512:(hh + 1) * 512],
                                      sps2[0:1, :])
            # transpose the denominator row back onto partitions on PE
            # (8 tiny [1,128] transposes; kills the HBM roundtrip)
            q_hbm = b * L + qc * 1024
            rcps = psX.tile([128, 8], F32, tag="x", name=f"rcps{b}_{qc}")
            for j in range(8):
                nc.tensor.transpose(rcps[:, j:j + 1],
                                    sums_sb[0:1, j * 128:(j + 1) * 128],
                                    ident1[:])
            rcols = recip_pool.tile([128, 8], F32, tag="rcols",
                                    name=f"rcols{b}_{qc}")
            nc.vector.reciprocal(rcols[:], rcps[:])
            ctr = psTr.tile([128, 8, 128], BF, tag="tr",
                            name=f"ctr{b}_{qc}")
            for j in range(8):
                nc.tensor.transpose(
                    ctr[:, j, :],
                    ctxT_sb[:, qc * 1024 + j * 128: qc * 1024 + (j + 1) * 128],
                    ident[:])
            nrm = norm_pool.tile([128, 8, DH], BF, tag="nrm",
                                 name=f"nrm{b}_{qc}")
            for j in range(8):
                nc.vector.tensor_scalar_mul(nrm[:, j, :], ctr[:, j, :],
                                            rcols[:, j:j + 1])
            nc.sync.dma_start(
                out=bass.AP(tensor=a2a_in[b].ap().tensor,
                            offset=qc * 1024 * DH,
                            ap=[[DH, 128], [128 * DH, 8], [1, DH]]),
                in_=nrm[:])

        # ---- phase-C weights live in root-carved pools; big loads are
        # emitted at batch boundaries so they never clog the SP queue ----
        wff_sb = wC_pool.tile([128, DKC, D], BF)
        wc1_sb = wC_pool.tile([128, DKC, D], BF)
        wc2_sb = wC_pool.tile([128, DKC], BF)
        bc1_sb = wC_pool.tile([128, DKC], F32)
        bc2_sb = wC_pool.tile([1, 1], F32)
        if not trivial_gb:
            gamma_bc = wC_pool.tile([128, D], BF)
            beta_bc = wC_pool.tile([128, D], BF)
        bffr = wC_pool.tile([1, D], BF)
        wl1r_sb = finW.tile([128, 2, L], BF)
        wl2_sb = finW.tile([128, L // 128, OUT], BF)
        bl1t_sb = finW.tile([128, L // 128, B], BF)
        bl2_bc = finW.tile([B, OUT], F32)

        def emit_small_weights():
            nc.sync.dma_start(
                out=wc2_sb[:],
                in_=bass.AP(tensor=wc2.ap().tensor, offset=0,
                            ap=[[1, 128], [128, DKC]]))
            nc.sync.dma_start(
                out=bc1_sb[:],
                in_=bass.AP(tensor=bc1.ap().tensor, offset=0,
                            ap=[[1, 128], [128, DKC]]))
            nc.sync.dma_start(out=bc2_sb[:], in_=bc2.ap())
            if not trivial_gb:
                nc.sync.dma_start(out=gamma_bc[:], in_=bcast(gamma, 128, D))
                nc.sync.dma_start(out=beta_bc[:], in_=bcast(beta, 128, D))
            nc.sync.dma_start(out=bffr[:],
                              in_=bff.ap().rearrange("(o n) -> o n", o=1))
            nc.sync.dma_start(
                out=bl1t_sb[:],
                in_=bass.AP(tensor=bl1t4.ap().tensor, offset=0,
                            ap=[[B, 128], [128 * B, L // 128], [1, B]]))
            nc.sync.dma_start(out=bl2_bc[:], in_=bcast(bl2, B, OUT))

        def emit_big_weights(stage):
            if stage == 1:
                nc.sync.dma_start(
                    out=wff_sb[:],
                    in_=bass.AP(tensor=wff.ap().tensor, offset=0,
                                ap=[[D, 128], [128 * D, DKC], [1, D]]))
            elif stage == 2:
                nc.sync.dma_start(
                    out=wc1_sb[:],
                    in_=bass.AP(tensor=wc1.ap().tensor, offset=0,
                                ap=[[D, 128], [128 * D, DKC], [1, D]]))
            elif stage == 3:
                nc.sync.dma_start(
                    out=wl1r_sb[:],
                    in_=bass.AP(tensor=wl1r.ap().tensor, offset=0,
                                ap=[[L, 128], [128 * L, 2], [1, L]]))
                nc.sync.dma_start(
                    out=wl2_sb[:],
                    in_=bass.AP(tensor=wl2f.ap().tensor, offset=0,
                                ap=[[OUT, 128], [128 * OUT, L // 128], [1, OUT]]))

        # ---------------- merged phase A+B, per-batch pipeline ----------------
        emit_xt_load(0)
        emit_small_weights()
        for s in range(3):
            emit_qkv_stage(0, s)
        for b in range(B):
            if b + 1 < B:
                emit_xt_load(b + 1)
            if 1 <= b <= 3:
                emit_big_weights(b)
            ctxT_sb = ctxT_pool.tile([128, L], BF, tag="ctxT", name=f"ctxT{b}")
            pT0 = pT_pool.tile([128, KCB, 1024], F8, tag="pT", name=f"pT{b}_0")
            emit_scores(b, 0, pT0)
            if b + 1 < B:
                emit_qkv_stage(b + 1, 0)
            pT1 = pT_pool.tile([128, KCB, 1024], F8, tag="pT", name=f"pT{b}_1")
            cps2_q0 = [psX.tile([128, 512], F32, tag="x", name=f"cps{b}_0_{h}")
                       for h in range(2)]
            emit_scores_interleaved(b, 1, pT1, pT0, cps2_q0)
            if b + 1 < B:
                emit_qkv_stage(b + 1, 1)
            emit_den_epilogue(b, 0, pT0, cps2_q0, ctxT_sb)
            if b + 1 < B:
                emit_qkv_stage(b + 1, 2)
            cps2_q1 = [psX.tile([128, 512], F32, tag="x", name=f"cps{b}_1_{h}")
                       for h in range(2)]
            emit_ctx(b, pT1, cps2_q1)
            emit_den_epilogue(b, 1, pT1, cps2_q1, ctxT_sb)
            nc.gpsimd.collective_compute(
                "AllToAll", OP.bypass,
                ins=[a2a_in[b].ap()],
                outs=[a2a_out[b].ap()],
                replica_groups=RG)

        phAB.close()  # release qkv/pT space for phase C

        # ================= Phase C: row-parallel LN/FF/collapse =================
        with ExitStack() as phC:
            rowC = phC.enter_context(tc.tile_pool(name="rowC", bufs=4))
            h2T_pool = phC.enter_context(tc.tile_pool(name="h2T", bufs=1))
            psFF = phC.enter_context(tc.tile_pool(name="psFF", bufs=2, space="PSUM"))
            psC1 = phC.enter_context(tc.tile_pool(name="psC1", bufs=2, space="PSUM"))
            psTrC = phC.enter_context(tc.tile_pool(name="psTrC", bufs=2, space="PSUM"))
            psSm = phC.enter_context(tc.tile_pool(name="psSm", bufs=1, space="PSUM"))

            h2T_half = [h2T_pool.tile([128, DKC, RPC // 2], BF, name=f"h2Th{i}")
                        for i in range(2)]
            c2_sb = h2T_pool.tile([1, RPC], F32)
            c1T = h2T_pool.tile([128, DKC, RPC], BF)

            def layernorm_rows(src, dst, apply_gb):
                stats = rowC.tile([128, 2, nc.vector.BN_STATS_DIM], F32, tag="stats")
                for sg in range(2):
                    nc.vector.bn_stats(stats[:, sg, :], src[:, sg * 512:(sg + 1) * 512])
                mv = rowC.tile([128, nc.vector.BN_AGGR_DIM], F32, tag="mv")
                nc.vector.bn_aggr(mv[:], stats[:])
                sq = rowC.tile([128, 1], F32, tag="sq")
                nc.scalar.activation(sq[:], mv[:, 1:2], AF.Sqrt, bias=eps_sb[:], scale=1.0)
                rstd = rowC.tile([128, 1], F32, tag="rstd")
                nc.vector.reciprocal(rstd[:], sq[:])
                if apply_gb and not trivial_gb:
                    z = rowC.tile([128, D], BF, tag="zf")
                    nc.vector.tensor_scalar(z[:], src[:], mv[:, 0:1], rstd[:],
                                            op0=OP.subtract, op1=OP.mult)
                    zg = rowC.tile([128, D], BF, tag="zg")
                    nc.vector.tensor_mul(zg[:], z[:], gamma_bc[:])
                    nc.vector.tensor_add(dst[:], zg[:], beta_bc[:])
                else:
                    nc.vector.tensor_scalar(dst[:], src[:], mv[:, 0:1], rstd[:],
                                            op0=OP.subtract, op1=OP.mult)

            h1b_t = {}
            h1T_t = {}

            def emit_head(t):
                """loads + residual add + LN1 + XBAR transpose of h1."""
                b, e = t // 2, t % 2
                ctx_t = rowC.tile([128, H, DH], BF, tag="ctx_t")
                nc.sync.dma_start(
                    out=ctx_t[:],
                    in_=bass.AP(tensor=a2a_out[b].ap().tensor,
                                offset=e * 128 * DH,
                                ap=[[DH, 128], [LPC * DH, H], [1, DH]]))
                x_t = rowC.tile([128, D], BF, tag="x_t")
                nc.sync.dma_start(out=x_t[:], in_=xrows.ap()[t * 128:(t + 1) * 128, :])
                s_t = rowC.tile([128, D], BF, tag="s_t")
                nc.vector.tensor_add(s_t[:], x_t[:],
                                     ctx_t[:].rearrange("p h d -> p (h d)"))
                h1b = rowC.tile([128, D], BF, tag="h1b")
                layernorm_rows(s_t, h1b, apply_gb=True)
                h1T = rowC.tile([128, DKC, 128], BF, tag="h1T")
                if t < 4:
                    # A2A3 may still be in flight; XBAR transposes serialize
                    # with collectives, so early tiles transpose on PE
                    tps1 = psTrC.tile([128, DKC, 128], BF, tag="htr",
                                      name=f"h1tr{t}")
                    for kc in range(DKC):
                        nc.tensor.transpose(tps1[:, kc, :],
                                            h1b[:, kc * 128:(kc + 1) * 128],
                                            ident[:])
                    nc.vector.tensor_copy(h1T[:], tps1[:])
                else:
                    nc.scalar.dma_start_transpose(out=h1T[:], in_=h1b[:])
                h1b_t[t] = h1b
                h1T_t[t] = h1T

            def emit_body(t):
                """ff + residual + LN2 + XBAR transpose of h2 into h2T_half."""
                h1b, h1T = h1b_t.pop(t), h1T_t.pop(t)
                f_t = rowC.tile([128, D], BF, tag="f_t")
                for dc in range(2):
                    fps = psFF.tile([128, 512], F32, tag="fps")
                    for kc in range(DKC):
                        nc.tensor.matmul(fps[:], h1T[:, kc, :],
                                         wff_sb[:, kc, dc * 512:(dc + 1) * 512],
                                         start=(kc == 0), stop=False)
                    nc.tensor.matmul(fps[:], ones_row[:],
                                     bffr[0:1, dc * 512:(dc + 1) * 512],
                                     start=False, stop=True)
                    nc.scalar.activation(f_t[:, dc * 512:(dc + 1) * 512], fps[:],
                                         AF.Relu, bias=0.0, scale=1.0)
                s2_t = rowC.tile([128, D], BF, tag="s2_t")
                nc.vector.tensor_add(s2_t[:], h1b[:], f_t[:])
                h2b = rowC.tile([128, D], BF, tag="h2b")
                layernorm_rows(s2_t, h2b, apply_gb=False)
                if t < 4:
                    tps2 = psTrC.tile([128, DKC, 128], BF, tag="htr",
                                      name=f"h2tr{t}")
                    for kc in range(DKC):
                        nc.tensor.transpose(tps2[:, kc, :],
                                            h2b[:, kc * 128:(kc + 1) * 128],
                                            ident[:])
                    nc.vector.tensor_copy(
                        h2T_half[t // 4][:, :, (t % 4) * 128:(t % 4 + 1) * 128],
                        tps2[:])
                else:
                    nc.scalar.dma_start_transpose(
                        out=h2T_half[t // 4][:, :, (t % 4) * 128:(t % 4 + 1) * 128],
                        in_=h2b[:])

            def emit_c1(rc, fc):
                cps = psC1.tile([128, 512], F32, tag="c1ps", name=f"c1ps{rc}_{fc}")
                for kc in range(DKC):
                    nc.tensor.matmul(cps[:], wc1_sb[:, kc, fc * 128:(fc + 1) * 128],
                                     h2T_half[rc][:, kc, :],
                                     start=(kc == 0), stop=(kc == DKC - 1))
                nc.scalar.activation(c1T[:, fc, rc * 512:(rc + 1) * 512], cps[:],
                                     AF.Relu, bias=bc1_sb[:, fc:fc + 1], scale=1.0)

            def emit_c2(rc):
                c2ps = psSm.tile([1, 512], F32, tag="sm", name=f"c2ps{rc}")
                for kc in range(DKC):
                    nc.tensor.matmul(c2ps[:], wc2_sb[:, kc:kc + 1],
                                     c1T[:, kc, rc * 512:(rc + 1) * 512],
                                     start=(kc == 0), stop=(kc == DKC - 1))
                nc.scalar.activation(c2_sb[0:1, rc * 512:(rc + 1) * 512], c2ps[:],
                                     AF.Relu, bias=bc2_sb[0:1, :], scale=1.0)

            # 2-stage pipeline with c1 chunks as PE filler
            emit_head(0)
            emit_head(1)
            c1_sched = {3: [(0, 0), (0, 1)], 4: [(0, 2), (0, 3)],
                        5: [(0, 4), (0, 5)], 6: [(0, 6), (0, 7)],
                        7: [(1, 0), (1, 1)]}
            for t in range(RPC // 128):
                emit_body(t)
                if t + 2 < RPC // 128:
                    emit_head(t + 2)
                for rc, fc in c1_sched.get(t, []):
                    emit_c1(rc, fc)
            emit_c2(0)
            for fc in range(2, DKC):
                emit_c1(1, fc)
            emit_c2(1)

            # ---- tail: partial c @ wl1 on local rows, one AllReduce, then
            # every core redundantly computes the tiny l2 matmul ----
            ctps = psSm.tile([128, 2, B], F32, tag="sm", name="ctps")
            for b_ in range(B):
                for e in range(2):
                    # c2 block (b_, e) -> column e-major so the l1 matmuls
                    # can take a contiguous [128, B] slice per e
                    nc.tensor.transpose(
                        ctps[:, e, b_:b_ + 1],
                        c2_sb[0:1, (b_ * 2 + e) * 128:(b_ * 2 + e + 1) * 128],
                        ident1[:])
            cT_sb = rowC.tile([128, 2, B], BF, tag="cT_sb")
            nc.vector.tensor_copy(cT_sb[:], ctps[:])
            l1ps = psSm.tile([128, L // 128, B], F32, tag="sm", name="l1ps")
            for j in range(L // 128):
                for e in range(2):
                    nc.tensor.matmul(l1ps[:, j, :],
                                     wl1r_sb[:, e, j * 128:(j + 1) * 128],
                                     cT_sb[:, e, :],
                                     start=(e == 0), stop=(e == 1))
            l1p_sb = rowC.tile([128, L // 128, B], BF, tag="l1p_sb")
            nc.vector.tensor_copy(l1p_sb[:], l1ps[:])
            l1p_ap = bass.AP(tensor=l1p_in.ap().tensor, offset=0,
                             ap=[[B, 128], [128 * B, L // 128], [1, B]])
            nc.sync.dma_start(out=l1p_ap, in_=l1p_sb[:])
            nc.gpsimd.collective_compute(
                "AllReduce", OP.add,
                ins=[l1p_in.ap()], outs=[l1p_out.ap()], replica_groups=RG)
            arT_sb = rowC.tile([128, L // 128, B], BF, tag="arT_sb")
            nc.sync.dma_start(
                out=arT_sb[:],
                in_=bass.AP(tensor=l1p_out.ap().tensor, offset=0,
                            ap=[[B, 128], [128 * B, L // 128], [1, B]]))
            l1b_sb = rowC.tile([128, L // 128, B], BF, tag="l1b_sb")
            nc.vector.tensor_add(l1b_sb[:], arT_sb[:], bl1t_sb[:])
            c1fT = rowC.tile([128, L // 128, B], BF, tag="c1fT")
            nc.vector.tensor_scalar_max(c1fT[:], l1b_sb[:], 0.0)
            ops = psSm.tile([B, OUT], F32, tag="sm", name="finps")
            for j in range(L // 128):
                nc.tensor.matmul(ops[:], c1fT[:, j, :], wl2_sb[:, j, :],
                                 start=(j == 0), stop=(j == L // 128 - 1))
            out_f = rowC.tile([B, OUT], F32, tag="out_f")
            nc.vector.tensor_add(out_f[:], ops[:], bl2_bc[:])
            nc.sync.dma_start(out=out.ap(), in_=out_f[:])

    nc.compile()
    return nc


def _to_bf16(a):
    return np.asarray(a, dtype=np.float32).astype(ml_dtypes.bfloat16)


def _to_f8(a):
    return np.asarray(a, dtype=np.float32).astype(ml_dtypes.float8_e4m3)


def kernel(**inputs):
    from concourse.bass_utils import run_bass_kernel_spmd

    gamma_np0 = np.asarray(inputs["gamma"], dtype=np.float32)
    beta_np0 = np.asarray(inputs["beta"], dtype=np.float32)
    trivial_gb = bool(np.all(gamma_np0 == 1.0) and np.all(beta_np0 == 0.0))
    key = ("nc", trivial_gb)
    if key not in _CACHE:
        _CACHE[key] = _build_nc(trivial_gb=trivial_gb)
    nc = _CACHE[key]

    x = np.asarray(inputs["x"], dtype=np.float32).reshape(N, D)
    isq = 1.0 / math.sqrt(DH)
    gamma_np = np.asarray(inputs["gamma"], dtype=np.float32)
    beta_np = np.asarray(inputs["beta"], dtype=np.float32)
    wc1_np = np.asarray(inputs["wc1"], dtype=np.float32)
    bc1_np = np.asarray(inputs["bc1"], dtype=np.float32)
    # fold LN2's gamma/beta into the c1 projection (h2 feeds only this matmul)
    wc1_f = gamma_np[:, None] * wc1_np
    bc1_f = bc1_np + beta_np @ wc1_np

    xT_f8 = np.ascontiguousarray(_to_f8(x).T)
    shared = dict(
        xT=xT_f8,
        wff=_to_bf16(inputs["wff"]),
        bff=_to_bf16(inputs["bff"]),
        gamma=_to_bf16(gamma_np), beta=_to_bf16(beta_np),
        wc1=_to_bf16(wc1_f), bc1=bc1_f.astype(np.float32),
        wc2=_to_bf16(np.asarray(inputs["wc2"]).reshape(D)),
        bc2=np.asarray(inputs["bc2"], np.float32).reshape(1),
        bl2=np.asarray(inputs["bl2"], np.float32),
    )
    wl1_np = np.asarray(inputs["wl1"], np.float32)
    bl1_np = np.asarray(inputs["bl1"], np.float32)
    wl2_np = np.asarray(inputs["wl2"], np.float32)
    shared["bl1t4"] = _to_bf16(np.repeat(bl1_np[:, None], B, axis=1))
    shared["wl2f"] = _to_bf16(wl2_np)
    # fp8 weights pre-scaled by SCL so they sit in e4m3's normal range;
    # the scale is undone by ESC in exp() and the SCL-valued ones-vector
    wq = np.asarray(inputs["wq"], np.float32) * (isq * SCL)
    bq = np.asarray(inputs["bq"], np.float32) * (isq * SCL)
    wk = np.asarray(inputs["wk"], np.float32) * SCL
    bk = np.asarray(inputs["bk"], np.float32) * SCL
    wv = np.asarray(inputs["wv"], np.float32) * SCL
    bv = np.asarray(inputs["bv"], np.float32)

    in_maps = []
    for i in range(NC):
        sl = slice(i * DH, (i + 1) * DH)
        wqkv_i = np.stack([wq[:, sl], wk[:, sl], wv[:, sl]])
        bqkv_i = np.stack([bq[sl], bk[sl], np.zeros_like(bk[sl])])
        # rows this core owns after the A2A; v-bias folded into x here
        xr = np.concatenate([
            x[b * L + i * LPC: b * L + (i + 1) * LPC, :] for b in range(B)
        ]) + bv[None, :]
        in_maps.append(dict(
            shared,
            wqkv=_to_f8(wqkv_i),
            bqkv=bqkv_i.astype(np.float32),
            xrows=_to_bf16(xr),
            wl1r=_to_bf16(wl1_np[i * LPC:(i + 1) * LPC, :]),
        ))

    res = run_bass_kernel_spmd(nc, in_maps, core_ids=list(range(NC)))
    return np.asarray(res.results[0]["out"], dtype=np.float32)


# revision 26
# speedup vs baseline: 1.0396x; 1.0192x over previous
"""Distributed Trainium2 kernel for nn_Attention_64854006169830.

Strategy (8 NeuronCores, SPMD):
  - Head-parallel attention (core i owns head i), feature-major activations.
  - QKV projections in fp8-e4m3 DoubleRow mode (2x PE); weights pre-scaled
    by 32 on host (e4m3 normal range), scale folded out via the exp() scale
    and the SCL-valued ones-vector in the denominators.
  - Per-batch software pipeline: QKV(b+1) stages are woven into
    attention(b) emission so the PE fills the gaps while the ACT engine
    (softmax exp, the phase bottleneck) streams.
  - Softmax: exp on ACT writes shifted unnormalized probabilities straight
    to fp8; denominators and attn@V are fp8 DoubleRow matmuls; all
    SBUF-side transposes ride the DMA XBAR (dma_start_transpose), not PE.
  - ctx redistribution head-shard -> row-shard via per-batch AllToAll.
  - Row-parallel LN/FF/collapse in bf16, emitted as a 2-stage pipeline
    (head=LN1, body=FF+LN2) with c1 chunks interleaved to keep PE fed.
    FF bias is applied by a rank-1 ones-row matmul into the same PSUM
    accumulation, so ACT applies relu straight from PSUM.
  - Tail: per-core partial c @ wl1 (row-sharded), one bf16 AllReduce, then
    every core redundantly computes the tiny l2 matmul.
Compute dtype: bf16/fp8-e4m3 (f32 accumulation); ~0.6% rel err vs the
float32 reference (gate is 2e-2).
"""
import sys
import math

for _p in ("/opt/trn_rl_repo", "/opt/trn_rl_repo/concourse"):
    if _p not in sys.path:
        sys.path.insert(0, _p)

import numpy as np
import ml_dtypes

B, L, D, H, OUT = 4, 2048, 1024, 8, 256
DH = D // H          # 128
N = B * L            # 8192 rows
NC = 8               # cores
RPC = N // NC        # 1024 rows per core (as 4 batches x 256 L-positions)
LPC = L // NC        # 256 L-positions per core per batch
EPS = 1e-12
SCL = 32.0           # fp8 weight pre-scale (host); folded out on device
C_SHIFT = 2.0        # softmax exp shift; cancels in normalization
BIAS_MM = True       # ff bias via rank-1 matmul instead of DVE add

_CACHE = {}


def _build_nc(trivial_gb=False):
    import concourse.bass as bass
    import concourse.tile as tile
    from concourse import bacc, mybir
    from concourse.masks import make_identity

    BF = mybir.dt.bfloat16
    F8 = mybir.dt.float8e4
    F32 = mybir.dt.float32
    AF = mybir.ActivationFunctionType
    OP = mybir.AluOpType
    DR = mybir.MatmulPerfMode.DoubleRow

    nc = bacc.Bacc("TRN2", debug=False, num_devices=NC)

    # ---- parameters (per-core values supplied via in_maps) ----
    xT = nc.dram_tensor("xT", [D, N], F8, kind="ExternalInput")
    xrows = nc.dram_tensor("xrows", [RPC, D], BF, kind="ExternalInput")
    wqkv = nc.dram_tensor("wqkv", [3, D, DH], F8, kind="ExternalInput")
    bqkv = nc.dram_tensor("bqkv", [3, DH], F32, kind="ExternalInput")
    wff = nc.dram_tensor("wff", [D, D], BF, kind="ExternalInput")
    bff = nc.dram_tensor("bff", [D], BF, kind="ExternalInput")
    gamma = nc.dram_tensor("gamma", [D], BF, kind="ExternalInput")
    beta = nc.dram_tensor("beta", [D], BF, kind="ExternalInput")
    wc1 = nc.dram_tensor("wc1", [D, D], BF, kind="ExternalInput")   # gamma-folded
    bc1 = nc.dram_tensor("bc1", [D], F32, kind="ExternalInput")     # beta-folded
    wc2 = nc.dram_tensor("wc2", [D], BF, kind="ExternalInput")
    bc2 = nc.dram_tensor("bc2", [1], F32, kind="ExternalInput")
    wl1r = nc.dram_tensor("wl1r", [LPC, L], BF, kind="ExternalInput")  # row-shard
    bl1t4 = nc.dram_tensor("bl1t4", [L, B], BF, kind="ExternalInput")
    wl2f = nc.dram_tensor("wl2f", [L, OUT], BF, kind="ExternalInput")  # full
    bl2 = nc.dram_tensor("bl2", [OUT], F32, kind="ExternalInput")
    out = nc.dram_tensor("out", [B, OUT], F32, kind="ExternalOutput")

    # ---- internal DRAM ----
    a2a_in = [nc.dram_tensor(f"a2a_in{b}", [L, DH], BF) for b in range(B)]
    a2a_out = [nc.dram_tensor(f"a2a_out{b}", [L, DH], BF) for b in range(B)]
    sums_hbm = nc.dram_tensor("sums_hbm", [N], F32)
    c_hbm = nc.dram_tensor("c_hbm", [RPC], BF)
    l1p_in = nc.dram_tensor("l1p_in", [L, B], BF)
    l1p_out = nc.dram_tensor("l1p_out", [L, B], BF, addr_space="Shared")

    def bcast(dram_handle, parts, free):
        """Broadcast a [free] DRAM vector across `parts` partitions."""
        ap = dram_handle.ap()
        return bass.AP(tensor=ap.tensor, offset=0, ap=[[0, parts], [1, free]])

    RG = [list(range(NC))]
    KCB = L // 128   # 16 key chunks per batch
    DKC = D // 128   # 8
    ESC = 1.0 / (SCL * SCL)

    from contextlib import ExitStack

    with tile.TileContext(nc) as tc, ExitStack() as root:
        glob = root.enter_context(tc.tile_pool(name="glob", bufs=1))
        ones2 = glob.tile([128, 2, 128], F8)
        nc.vector.memset(ones2[:], SCL)  # folds the v-scale back out of ctx
        eps_sb = glob.tile([128, 1], F32)
        nc.vector.memset(eps_sb[:], EPS)
        negc_sb = glob.tile([128, 1], F32)
        nc.vector.memset(negc_sb[:], -C_SHIFT)
        ones_row = glob.tile([1, 128], BF)
        nc.vector.memset(ones_row[:], 1.0)
        ident1 = glob.tile([1, 1], F32)
        nc.vector.memset(ident1[:], 1.0)
        ident = glob.tile([128, 128], BF)
        make_identity(nc, ident[:])

        # Phase-C weight pool carved out first so its loads never overlap
        # (in address space) with the big transient phase-A/B tiles.
        wC_pool = root.enter_context(tc.tile_pool(name="wC", bufs=1))
        finW = root.enter_context(tc.tile_pool(name="finW", bufs=1))

        phAB = root.enter_context(ExitStack())
        qkv_pool = phAB.enter_context(tc.tile_pool(name="qkv", bufs=1))
        qkvT = [qkv_pool.tile([128, 2, L], BF, name=f"qkvT{b}") for b in range(B)]
        vnat = [qkv_pool.tile([128, L // 128, DH], F8, name=f"vnat{b}")
                for b in range(B)]

        xt_pool = phAB.enter_context(tc.tile_pool(name="xt", bufs=2))
        wq_pool = phAB.enter_context(tc.tile_pool(name="wqkv", bufs=1))
        vstage_pool = phAB.enter_context(tc.tile_pool(name="vstage", bufs=2))
        pT_pool = phAB.enter_context(tc.tile_pool(name="pT", bufs=2))
        ctxT_pool = phAB.enter_context(tc.tile_pool(name="ctxT", bufs=2))
        sums_pool = phAB.enter_context(tc.tile_pool(name="sums", bufs=1))
        recip_pool = phAB.enter_context(tc.tile_pool(name="recip", bufs=2))
        norm_pool = phAB.enter_context(tc.tile_pool(name="norm", bufs=3))
        psA = phAB.enter_context(tc.tile_pool(name="psA", bufs=1, space="PSUM"))
        psTr = phAB.enter_context(tc.tile_pool(name="psTr", bufs=1, space="PSUM"))
        psS = phAB.enter_context(tc.tile_pool(name="psS", bufs=2, space="PSUM"))
        psX = phAB.enter_context(tc.tile_pool(name="psX", bufs=2, space="PSUM"))

        wq_sb = wq_pool.tile([128, 3, DKC, DH], F8)
        nc.sync.dma_start(
            out=wq_sb[:],
            in_=bass.AP(tensor=wqkv.ap().tensor, offset=0,
                        ap=[[DH, 128], [D * DH, 3], [128 * DH, DKC], [1, DH]]))
        bq_sb = wq_pool.tile([128, 3], F32)
        nc.sync.dma_start(
            out=bq_sb[:],
            in_=bass.AP(tensor=bqkv.ap().tensor, offset=0,
                        ap=[[1, 128], [DH, 3]]))

        xt_tiles = {}
        xt_last_dma = None

        def emit_xt_load(b):
            xt = xt_pool.tile([128, DKC, L], F8, tag="xt", name=f"xt{b}")
            xt_tiles[b] = xt
            nonlocal xt_last_dma
            for kc in range(DKC):
                xt_last_dma = nc.sync.dma_start(
                    out=xt[:, kc, :],
                    in_=xT.ap()[kc * 128:(kc + 1) * 128,
                                b * 2048:(b + 1) * 2048])

        def emit_qkv_stage(b, s):
            """One projection (q/k/v) of batch b: 4 DR-matmul chains + epilogue."""
            xt = xt_tiles[b]
            for r4 in range(4):
                pst = psA.tile([128, 512], F32, tag="qkvps",
                               name=f"qkvps{b}_{s}_{r4}")
                for k2 in range(D // 256):
                    nc.tensor.matmul(
                        pst[:], wq_sb[:, s, 2 * k2:2 * k2 + 2, :],
                        xt[:, 2 * k2:2 * k2 + 2, r4 * 512:(r4 + 1) * 512],
                        start=(k2 == 0), stop=(k2 == D // 256 - 1),
                        perf_mode=DR)
                if s < 2:
                    nc.vector.tensor_scalar_add(
                        qkvT[b][:, s, r4 * 512:(r4 + 1) * 512], pst[:],
                        bq_sb[:, s:s + 1])
                else:
                    # v (bias folded into xrows on host): psum -> bf16 staging,
                    # XBAR transpose to row-major, then fp8 for the DR matmuls
                    vstage = vstage_pool.tile([128, 512], BF, tag="vstage",
                                              name=f"vst{b}_{r4}")
                    nc.vector.tensor_copy(vstage[:], pst[:])
                    vtr = psTr.tile([128, 4, 128], BF, tag="tr",
                                    name=f"vtr{b}_{r4}")
                    for j in range(4):
                        nc.tensor.transpose(vtr[:, j, :],
                                            vstage[:, j * 128:(j + 1) * 128],
                                            ident[:])
                    nc.vector.tensor_copy(vnat[b][:, r4 * 4:(r4 + 1) * 4, :], vtr[:])

        def emit_scores(b, qc, pT):
            q0 = qc * 1024
            for kc in range(KCB):
                sps = psS.tile([128, 1024], F32, tag="sps",
                               name=f"sps{b}_{qc}_{kc}")
                for hh in range(2):
                    nc.tensor.matmul(
                        sps[:, hh * 512:(hh + 1) * 512],
                        qkvT[b][:, 1, kc * 128:(kc + 1) * 128],
                        qkvT[b][:, 0, q0 + hh * 512: q0 + (hh + 1) * 512],
                        start=True, stop=True)
                nc.scalar.activation(pT[:, kc, :], sps[:], AF.Exp,
                                     bias=negc_sb[:], scale=ESC)

        def emit_scores_interleaved(b, qc, pT, pT_prev, cps_prev):
            """scores/exp for (b,qc) with ctx DR-matmuls of the previous qc
            woven in as the needed pT chunks complete."""
            q0 = qc * 1024
            for kc in range(KCB):
                sps = psS.tile([128, 1024], F32, tag="sps",
                               name=f"sps{b}_{qc}_{kc}i")
                for hh in range(2):
                    nc.tensor.matmul(
                        sps[:, hh * 512:(hh + 1) * 512],
                        qkvT[b][:, 1, kc * 128:(kc + 1) * 128],
                        qkvT[b][:, 0, q0 + hh * 512: q0 + (hh + 1) * 512],
                        start=True, stop=True)
                nc.scalar.activation(pT[:, kc, :], sps[:], AF.Exp,
                                     bias=negc_sb[:], scale=ESC)
                if kc % 2 == 1:
                    k2 = kc // 2
                    for hh in range(2):
                        nc.tensor.matmul(
                            cps_prev[hh][:],
                            vnat[b][:, 2 * k2:2 * k2 + 2, :],
                            pT_prev[:, 2 * k2:2 * k2 + 2,
                                    hh * 512:(hh + 1) * 512],
                            start=(k2 == 0), stop=(k2 == KCB // 2 - 1),
                            perf_mode=DR)

        def emit_ctx(b, pT, cps2):
            for k2 in range(KCB // 2):
                for hh in range(2):
                    nc.tensor.matmul(
                        cps2[hh][:], vnat[b][:, 2 * k2:2 * k2 + 2, :],
                        pT[:, 2 * k2:2 * k2 + 2, hh * 512:(hh + 1) * 512],
                        start=(k2 == 0), stop=(k2 == KCB // 2 - 1),
                        perf_mode=DR)

        def emit_den_epilogue(b, qc, pT, cps2, ctxT_sb):
            """ctxT copies, DR-ones denominators, recip roundtrip, XBAR
            transpose back to row-major, normalize, ship to a2a_in."""
            for hh in range(2):
                nc.vector.tensor_copy(
                    ctxT_sb[:, qc * 1024 + hh * 512: qc * 1024 + (hh + 1) * 512],
                    cps2[hh][:])
            sums_sb = sums_pool.tile([1, 1024], F32, tag="sums", bufs=2,
                                     name=f"sums{b}_{qc}")
            for hh in range(2):
                sps2 = psX.tile([128, 512], F32, tag="x", name=f"den{b}_{qc}_{hh}")
                for k2 in range(KCB // 2):
                    nc.tensor.matmul(sps2[:], ones2[:],
                                     pT[:, 2 * k2:2 * k2 + 2,
                                        hh * 512:(hh + 1) * 512],
                                     start=(k2 == 0), stop=(k2 == KCB // 2 - 1),
                                     perf_mode=DR)
                nc.vector.tensor_copy(sums_sb[:, hh * 512:(hh + 1) * 512],
                                      sps2[0:1, :])
            # transpose the denominator row back onto partitions on PE
            # (8 tiny [1,128] transposes; kills the HBM roundtrip)
            q_hbm = b * L + qc * 1024
            rcps = psX.tile([128, 8], F32, tag="x", name=f"rcps{b}_{qc}")
            for j in range(8):
                nc.tensor.transpose(rcps[:, j:j + 1],
                                    sums_sb[0:1, j * 128:(j + 1) * 128],
                                    ident1[:])
            rcols = recip_pool.tile([128, 8], F32, tag="rcols",
                                    name=f"rcols{b}_{qc}")
            nc.vector.reciprocal(rcols[:], rcps[:])
            ctr = psTr.tile([128, 8, 128], BF, tag="tr",
                            name=f"ctr{b}_{qc}")
            for j in range(8):
                nc.tensor.transpose(
                    ctr[:, j, :],
                    ctxT_sb[:, qc * 1024 + j * 128: qc * 1024 + (j + 1) * 128],
                    ident[:])
            nrm = norm_pool.tile([128, 8, DH], BF, tag="nrm",
                                 name=f"nrm{b}_{qc}")
            for j in range(8):
                nc.vector.tensor_scalar_mul(nrm[:, j, :], ctr[:, j, :],
                                            rcols[:, j:j + 1])
            nc.sync.dma_start(
                out=bass.AP(tensor=a2a_in[b].ap().tensor,
                            offset=qc * 1024 * DH,
                            ap=[[DH, 128], [128 * DH, 8], [1, DH]]),
                in_=nrm[:])

        # ---- phase-C weights live in root-carved pools; big loads are
        # emitted at batch boundaries so they never clog the SP queue ----
        wff_sb = wC_pool.tile([128, DKC, D], BF)
        wc1_sb = wC_pool.tile([128, DKC, D], BF)
        wc2_sb = wC_pool.tile([128, DKC], BF)
        bc1_sb = wC_pool.tile([128, DKC], F32)
        bc2_sb = wC_pool.tile([1, 1], F32)
        if not trivial_gb:
            gamma_bc = wC_pool.tile([128, D], BF)
            beta_bc = wC_pool.tile([128, D], BF)
        bffr = wC_pool.tile([1, D], BF)
        wl1r_sb = finW.tile([128, 2, L], BF)
        wl2_sb = finW.tile([128, L // 128, OUT], BF)
        bl1t_sb = finW.tile([128, L // 128, B], BF)
        bl2_bc = finW.tile([B, OUT], F32)

        def emit_small_weights():
            nc.sync.dma_start(
                out=wc2_sb[:],
                in_=bass.AP(tensor=wc2.ap().tensor, offset=0,
                            ap=[[1, 128], [128, DKC]]))
            nc.sync.dma_start(
                out=bc1_sb[:],
                in_=bass.AP(tensor=bc1.ap().tensor, offset=0,
                            ap=[[1, 128], [128, DKC]]))
            nc.sync.dma_start(out=bc2_sb[:], in_=bc2.ap())
            if not trivial_gb:
                nc.sync.dma_start(out=gamma_bc[:], in_=bcast(gamma, 128, D))
                nc.sync.dma_start(out=beta_bc[:], in_=bcast(beta, 128, D))
            nc.sync.dma_start(out=bffr[:],
                              in_=bff.ap().rearrange("(o n) -> o n", o=1))
            nc.sync.dma_start(
                out=bl1t_sb[:],
                in_=bass.AP(tensor=bl1t4.ap().tensor, offset=0,
                            ap=[[B, 128], [128 * B, L // 128], [1, B]]))
            nc.sync.dma_start(out=bl2_bc[:], in_=bcast(bl2, B, OUT))

        def emit_big_weights(stage):
            if stage == 1:
                nc.sync.dma_start(
                    out=wff_sb[:],
                    in_=bass.AP(tensor=wff.ap().tensor, offset=0,
                                ap=[[D, 128], [128 * D, DKC], [1, D]]))
            elif stage == 2:
                nc.sync.dma_start(
                    out=wc1_sb[:],
                    in_=bass.AP(tensor=wc1.ap().tensor, offset=0,
                                ap=[[D, 128], [128 * D, DKC], [1, D]]))
            elif stage == 3:
                nc.sync.dma_start(
                    out=wl1r_sb[:],
                    in_=bass.AP(tensor=wl1r.ap().tensor, offset=0,
                                ap=[[L, 128], [128 * L, 2], [1, L]]))
                nc.sync.dma_start(
                    out=wl2_sb[:],
                    in_=bass.AP(tensor=wl2f.ap().tensor, offset=0,
                                ap=[[OUT, 128], [128 * OUT, L // 128], [1, OUT]]))

        # ---------------- merged phase A+B, per-batch pipeline ----------------
        emit_xt_load(0)
        emit_small_weights()
        for s in range(3):
            emit_qkv_stage(0, s)
        for b in range(B):
            if b + 1 < B:
                emit_xt_load(b + 1)
            if 1 <= b <= 3:
                emit_big_weights(b)
            ctxT_sb = ctxT_pool.tile([128, L], BF, tag="ctxT", name=f"ctxT{b}")
            pT0 = pT_pool.tile([128, KCB, 1024], F8, tag="pT", name=f"pT{b}_0")
            emit_scores(b, 0, pT0)
            if b + 1 < B:
                emit_qkv_stage(b + 1, 0)
            pT1 = pT_pool.tile([128, KCB, 1024], F8, tag="pT", name=f"pT{b}_1")
            cps2_q0 = [psX.tile([128, 512], F32, tag="x", name=f"cps{b}_0_{h}")
                       for h in range(2)]
            emit_scores_interleaved(b, 1, pT1, pT0, cps2_q0)
            if b + 1 < B:
                emit_qkv_stage(b + 1, 1)
            emit_den_epilogue(b, 0, pT0, cps2_q0, ctxT_sb)
            if b + 1 < B:
                emit_qkv_stage(b + 1, 2)
            cps2_q1 = [psX.tile([128, 512], F32, tag="x", name=f"cps{b}_1_{h}")
                       for h in range(2)]
            emit_ctx(b, pT1, cps2_q1)
            emit_den_epilogue(b, 1, pT1, cps2_q1, ctxT_sb)
            nc.gpsimd.collective_compute(
                "AllToAll", OP.bypass,
                ins=[a2a_in[b].ap()],
                outs=[a2a_out[b].ap()],
                replica_groups=RG)

        phAB.close()  # release qkv/pT space for phase C

        # ================= Phase C: row-parallel LN/FF/collapse =================
        with ExitStack() as phC:
            rowC = phC.enter_context(tc.tile_pool(name="rowC", bufs=4))
            h2T_pool = phC.enter_context(tc.tile_pool(name="h2T", bufs=1))
            psFF = phC.enter_context(tc.tile_pool(name="psFF", bufs=2, space="PSUM"))
            psC1 = phC.enter_context(tc.tile_pool(name="psC1", bufs=2, space="PSUM"))
            psTrC = phC.enter_context(tc.tile_pool(name="psTrC", bufs=2, space="PSUM"))
            psSm = phC.enter_context(tc.tile_pool(name="psSm", bufs=1, space="PSUM"))
            psL1 = phC.enter_context(tc.tile_pool(name="psL1", bufs=1, space="PSUM"))

            h2T_half = [h2T_pool.tile([128, DKC, RPC // 2], BF, name=f"h2Th{i}")
                        for i in range(2)]
            c2_sb = h2T_pool.tile([1, RPC], F32)
            c1T = h2T_pool.tile([128, DKC, RPC], BF)

            def layernorm_rows(src, dst, apply_gb):
                stats = rowC.tile([128, 2, nc.vector.BN_STATS_DIM], F32, tag="stats")
                for sg in range(2):
                    nc.vector.bn_stats(stats[:, sg, :], src[:, sg * 512:(sg + 1) * 512])
                mv = rowC.tile([128, nc.vector.BN_AGGR_DIM], F32, tag="mv")
                nc.vector.bn_aggr(mv[:], stats[:])
                sq = rowC.tile([128, 1], F32, tag="sq")
                nc.scalar.activation(sq[:], mv[:, 1:2], AF.Sqrt, bias=eps_sb[:], scale=1.0)
                rstd = rowC.tile([128, 1], F32, tag="rstd")
                nc.vector.reciprocal(rstd[:], sq[:])
                if apply_gb and not trivial_gb:
                    z = rowC.tile([128, D], BF, tag="zf")
                    nc.vector.tensor_scalar(z[:], src[:], mv[:, 0:1], rstd[:],
                                            op0=OP.subtract, op1=OP.mult)
                    zg = rowC.tile([128, D], BF, tag="zg")
                    nc.vector.tensor_mul(zg[:], z[:], gamma_bc[:])
                    nc.vector.tensor_add(dst[:], zg[:], beta_bc[:])
                else:
                    nc.vector.tensor_scalar(dst[:], src[:], mv[:, 0:1], rstd[:],
                                            op0=OP.subtract, op1=OP.mult)

            h1b_t = {}
            h1T_t = {}

            def emit_head(t):
                """loads + residual add + LN1 + XBAR transpose of h1."""
                b, e = t // 2, t % 2
                ctx_t = rowC.tile([128, H, DH], BF, tag="ctx_t")
                nc.sync.dma_start(
                    out=ctx_t[:],
                    in_=bass.AP(tensor=a2a_out[b].ap().tensor,
                                offset=e * 128 * DH,
                                ap=[[DH, 128], [LPC * DH, H], [1, DH]]))
                x_t = rowC.tile([128, D], BF, tag="x_t")
                nc.sync.dma_start(out=x_t[:], in_=xrows.ap()[t * 128:(t + 1) * 128, :])
                s_t = rowC.tile([128, D], BF, tag="s_t")
                nc.vector.tensor_add(s_t[:], x_t[:],
                                     ctx_t[:].rearrange("p h d -> p (h d)"))
                h1b = rowC.tile([128, D], BF, tag="h1b")
                layernorm_rows(s_t, h1b, apply_gb=True)
                h1T = rowC.tile([128, DKC, 128], BF, tag="h1T")
                if t < 4:
                    # A2A3 may still be in flight; XBAR transposes serialize
                    # with collectives, so early tiles transpose on PE
                    tps1 = psTrC.tile([128, DKC, 128], BF, tag="htr",
                                      name=f"h1tr{t}")
                    for kc in range(DKC):
                        nc.tensor.transpose(tps1[:, kc, :],
                                            h1b[:, kc * 128:(kc + 1) * 128],
                                            ident[:])
                    nc.vector.tensor_copy(h1T[:], tps1[:])
                else:
                    nc.scalar.dma_start_transpose(out=h1T[:], in_=h1b[:])
                h1b_t[t] = h1b
                h1T_t[t] = h1T

            def emit_body(t):
                """ff + residual + LN2 + XBAR transpose of h2 into h2T_half."""
                h1b, h1T = h1b_t.pop(t), h1T_t.pop(t)
                f_t = rowC.tile([128, D], BF, tag="f_t")
                for dc in range(2):
                    fps = psFF.tile([128, 512], F32, tag="fps")
                    for kc in range(DKC):
                        nc.tensor.matmul(fps[:], h1T[:, kc, :],
                                         wff_sb[:, kc, dc * 512:(dc + 1) * 512],
                                         start=(kc == 0), stop=False)
                    nc.tensor.matmul(fps[:], ones_row[:],
                                     bffr[0:1, dc * 512:(dc + 1) * 512],
                                     start=False, stop=True)
                    nc.scalar.activation(f_t[:, dc * 512:(dc + 1) * 512], fps[:],
                                         AF.Relu, bias=0.0, scale=1.0)
                s2_t = rowC.tile([128, D], BF, tag="s2_t")
                nc.vector.tensor_add(s2_t[:], h1b[:], f_t[:])
                h2b = rowC.tile([128, D], BF, tag="h2b")
                layernorm_rows(s2_t, h2b, apply_gb=False)
                if t < 4:
                    tps2 = psTrC.tile([128, DKC, 128], BF, tag="htr",
                                      name=f"h2tr{t}")
                    for kc in range(DKC):
                        nc.tensor.transpose(tps2[:, kc, :],
                                            h2b[:, kc * 128:(kc + 1) * 128],
                                            ident[:])
                    nc.vector.tensor_copy(
                        h2T_half[t // 4][:, :, (t % 4) * 128:(t % 4 + 1) * 128],
                        tps2[:])
                else:
                    nc.scalar.dma_start_transpose(
                        out=h2T_half[t // 4][:, :, (t % 4) * 128:(t % 4 + 1) * 128],
                        in_=h2b[:])

            def emit_c1(rc, fc):
                cps = psC1.tile([128, 512], F32, tag="c1ps", name=f"c1ps{rc}_{fc}")
                for kc in range(DKC):
                    nc.tensor.matmul(cps[:], wc1_sb[:, kc, fc * 128:(fc + 1) * 128],
                                     h2T_half[rc][:, kc, :],
                                     start=(kc == 0), stop=(kc == DKC - 1))
                nc.scalar.activation(c1T[:, fc, rc * 512:(rc + 1) * 512], cps[:],
                                     AF.Relu, bias=bc1_sb[:, fc:fc + 1], scale=1.0)

            def emit_c2(rc):
                c2ps = psSm.tile([1, 512], F32, tag="sm", name=f"c2ps{rc}")
                for kc in range(DKC):
                    nc.tensor.matmul(c2ps[:], wc2_sb[:, kc:kc + 1],
                                     c1T[:, kc, rc * 512:(rc + 1) * 512],
                                     start=(kc == 0), stop=(kc == DKC - 1))
                nc.scalar.activation(c2_sb[0:1, rc * 512:(rc + 1) * 512], c2ps[:],
                                     AF.Relu, bias=bc2_sb[0:1, :], scale=1.0)

            # 2-stage pipeline with c1 chunks as PE filler
            emit_head(0)
            emit_head(1)
            c1_sched = {3: [(0, 0), (0, 1)], 4: [(0, 2), (0, 3)],
                        5: [(0, 4), (0, 5)], 6: [(0, 6), (0, 7)],
                        7: [(1, 0), (1, 1)]}
            for t in range(RPC // 128):
                emit_body(t)
                if t + 2 < RPC // 128:
                    emit_head(t + 2)
                for rc, fc in c1_sched.get(t, []):
                    emit_c1(rc, fc)
            # ---- tail: partial c @ wl1 on local rows (computed in two
            # column-halves so the b01 half hides under c1/c2 of rows 512+),
            # ONE AllReduce, then the tiny l2 matmul on every core ----
            l1ps = psL1.tile([128, L // 128, B], F32, tag="l1", name="l1ps")

            def emit_l1_half(i):
                # half i: c2 rows of rc=i (batches 2i, 2i+1) -> out cols 2i:2i+2
                ctps = psSm.tile([128, 2, 2], F32, tag="sm", name=f"ctps{i}")
                for bb in range(2):
                    for e in range(2):
                        nc.tensor.transpose(
                            ctps[:, e, bb:bb + 1],
                            c2_sb[0:1, ((2 * i + bb) * 2 + e) * 128:
                                       ((2 * i + bb) * 2 + e + 1) * 128],
                            ident1[:])
                cT_sb = rowC.tile([128, 2, 2], BF, tag="cT_sb", name=f"cT{i}")
                nc.vector.tensor_copy(cT_sb[:], ctps[:])
                for j in range(L // 128):
                    for e in range(2):
                        nc.tensor.matmul(l1ps[:, j, 2 * i:2 * i + 2],
                                         wl1r_sb[:, e, j * 128:(j + 1) * 128],
                                         cT_sb[:, e, :],
                                         start=(e == 0), stop=(e == 1))

            emit_c2(0)
            emit_l1_half(0)
            for fc in range(2, DKC):
                emit_c1(1, fc)
            emit_c2(1)
            emit_l1_half(1)
            l1p_sb = rowC.tile([128, L // 128, B], BF, tag="l1p_sb")
            nc.vector.tensor_copy(l1p_sb[:], l1ps[:])
            l1p_ap = bass.AP(tensor=l1p_in.ap().tensor, offset=0,
                             ap=[[B, 128], [128 * B, L // 128], [1, B]])
            nc.sync.dma_start(out=l1p_ap, in_=l1p_sb[:])
            nc.gpsimd.collective_compute(
                "AllReduce", OP.add,
                ins=[l1p_in.ap()], outs=[l1p_out.ap()], replica_groups=RG)
            arT_sb = rowC.tile([128, L // 128, B], BF, tag="arT_sb")
            nc.sync.dma_start(
                out=arT_sb[:],
                in_=bass.AP(tensor=l1p_out.ap().tensor, offset=0,
                            ap=[[B, 128], [128 * B, L // 128], [1, B]]))
            l1b_sb = rowC.tile([128, L // 128, B], BF, tag="l1b_sb")
            nc.vector.tensor_add(l1b_sb[:], arT_sb[:], bl1t_sb[:])
            c1fT = rowC.tile([128, L // 128, B], BF, tag="c1fT")
            nc.vector.tensor_scalar_max(c1fT[:], l1b_sb[:], 0.0)
            ops = psSm.tile([B, OUT], F32, tag="sm", name="finps")
            for j in range(L // 128):
                nc.tensor.matmul(ops[:], c1fT[:, j, :], wl2_sb[:, j, :],
                                 start=(j == 0), stop=(j == L // 128 - 1))
            out_f = rowC.tile([B, OUT], F32, tag="out_f")
            nc.vector.tensor_add(out_f[:], ops[:], bl2_bc[:])
            nc.sync.dma_start(out=out.ap(), in_=out_f[:])

    nc.compile()
    return nc


def _to_bf16(a):
    return np.asarray(a, dtype=np.float32).astype(ml_dtypes.bfloat16)


def _to_f8(a):
    return np.asarray(a, dtype=np.float32).astype(ml_dtypes.float8_e4m3)


def kernel(**inputs):
    from concourse.bass_utils import run_bass_kernel_spmd

    gamma_np0 = np.asarray(inputs["gamma"], dtype=np.float32)
    beta_np0 = np.asarray(inputs["beta"], dtype=np.float32)
    trivial_gb = bool(np.all(gamma_np0 == 1.0) and np.all(beta_np0 == 0.0))
    key = ("nc", trivial_gb)
    if key not in _CACHE:
        _CACHE[key] = _build_nc(trivial_gb=trivial_gb)
    nc = _CACHE[key]

    x = np.asarray(inputs["x"], dtype=np.float32).reshape(N, D)
    isq = 1.0 / math.sqrt(DH)
    gamma_np = np.asarray(inputs["gamma"], dtype=np.float32)
    beta_np = np.asarray(inputs["beta"], dtype=np.float32)
    wc1_np = np.asarray(inputs["wc1"], dtype=np.float32)
    bc1_np = np.asarray(inputs["bc1"], dtype=np.float32)
    # fold LN2's gamma/beta into the c1 projection (h2 feeds only this matmul)
    wc1_f = gamma_np[:, None] * wc1_np
    bc1_f = bc1_np + beta_np @ wc1_np

    xT_f8 = np.ascontiguousarray(_to_f8(x).T)
    shared = dict(
        xT=xT_f8,
        wff=_to_bf16(inputs["wff"]),
        bff=_to_bf16(inputs["bff"]),
        gamma=_to_bf16(gamma_np), beta=_to_bf16(beta_np),
        wc1=_to_bf16(wc1_f), bc1=bc1_f.astype(np.float32),
        wc2=_to_bf16(np.asarray(inputs["wc2"]).reshape(D)),
        bc2=np.asarray(inputs["bc2"], np.float32).reshape(1),
        bl2=np.asarray(inputs["bl2"], np.float32),
    )
    wl1_np = np.asarray(inputs["wl1"], np.float32)
    bl1_np = np.asarray(inputs["bl1"], np.float32)
    wl2_np = np.asarray(inputs["wl2"], np.float32)
    shared["bl1t4"] = _to_bf16(np.repeat(bl1_np[:, None], B, axis=1))
    shared["wl2f"] = _to_bf16(wl2_np)
    # fp8 weights pre-scaled by SCL so they sit in e4m3's normal range;
    # the scale is undone by ESC in exp() and the SCL-valued ones-vector
    wq = np.asarray(inputs["wq"], np.float32) * (isq * SCL)
    bq = np.asarray(inputs["bq"], np.float32) * (isq * SCL)
    wk = np.asarray(inputs["wk"], np.float32) * SCL
    bk = np.asarray(inputs["bk"], np.float32) * SCL
    wv = np.asarray(inputs["wv"], np.float32) * SCL
    bv = np.asarray(inputs["bv"], np.float32)

    in_maps = []
    for i in range(NC):
        sl = slice(i * DH, (i + 1) * DH)
        wqkv_i = np.stack([wq[:, sl], wk[:, sl], wv[:, sl]])
        bqkv_i = np.stack([bq[sl], bk[sl], np.zeros_like(bk[sl])])
        # rows this core owns after the A2A; v-bias folded into x here
        xr = np.concatenate([
            x[b * L + i * LPC: b * L + (i + 1) * LPC, :] for b in range(B)
        ]) + bv[None, :]
        in_maps.append(dict(
            shared,
            wqkv=_to_f8(wqkv_i),
            bqkv=bqkv_i.astype(np.float32),
            xrows=_to_bf16(xr),
            wl1r=_to_bf16(wl1_np[i * LPC:(i + 1) * LPC, :]),
        ))

    res = run_bass_kernel_spmd(nc, in_maps, core_ids=list(range(NC)))
    return np.asarray(res.results[0]["out"], dtype=np.float32)
